# revision 1
# baseline (speedup 1.0000x reference)
"""GCNNet forward on 8 Trainium2 NeuronCores (Bass/Tile SPMD).

Strategy
--------
- Nodes partitioned graph-aligned across 8 cores (B/8 graphs per core).
- Per-core node slice processed in 128-node blocks. Edge aggregation
  (segment-sum with GCN symmetric norm, incl. self-loops) is computed as
  one-hot matmuls: S[slot, node] = norm, accumulated in PSUM over K tiles
  of 128 gathered source rows each (dma_gather, int16 idx, 4 row-ranges).
- Per layer aggregate at width min(din,dout): L1/L4/L5 aggregate-then-
  matmul, L2/L3 matmul-then-aggregate. All BatchNorm affine transforms and
  biases fold into weights host-side (rank-1 rows with r[n] = row-sum of
  norm, and ones). Only leaky-relu runs on the device (ACT engine).
- Cross-core exchange: 4 bf16 AllGathers of the next gather source.
- Attention pooling: per-graph one-hot matmuls into windows of 128 graphs,
  softmax without max-subtraction (pool unnormalized, divide by sum-exp),
  then the 3 FC layers per window. Host fixes empty graphs.
"""
import os
import sys

for _p in ("/opt/trn_rl_repo", "/root/.axon_site/_ro/trn_rl_repo"):
    if os.path.isdir(_p) and _p not in sys.path:
        sys.path.insert(0, _p)

import numpy as np
import ml_dtypes

import concourse.bass as bass
import concourse.bacc as bacc
import concourse.mybir as mybir
import concourse.tile as tile
from concourse.bass_utils import run_bass_kernel_spmd
from concourse.masks import make_identity

P = 128
NCORES = 8
NRANGE = 4
GBLK = 1  # blocks per gather group (HW dma_gather breaks with large multi-offset chunks)

bf16 = mybir.dt.float16  # working dtype (fp16: 10-bit mantissa, ranges are safe)
f32 = mybir.dt.float32
i16 = mybir.dt.int16
BF = np.float16

B_DEFAULT = 2048
PHASES = 5  # debug: how many phases of the program to emit
MAXG = 10**9  # debug: limit emitted gather groups in conv_pre


def set_f32_debug():
    """Switch all working dtypes to fp32 (slow; numeric debugging only)."""
    global bf16, BF
    bf16 = mybir.dt.float32
    BF = np.float32


def set_f16():
    global bf16, BF
    bf16 = mybir.dt.float16
    BF = np.float16


def _ceil(a, b):
    return -(-a // b)


# ----------------------------------------------------------------- host prep

def _preprocess(x, edge_index, edge_attr, batch, B):
    N = x.shape[0]
    GPC = B // NCORES
    src = np.asarray(edge_index[0], np.int64)
    dst = np.asarray(edge_index[1], np.int64)
    ew = np.asarray(edge_attr, np.float64)
    batch = np.asarray(batch, np.int64)

    gstarts = np.searchsorted(batch, np.arange(0, B + 1, GPC))
    node_start = gstarts[:-1]
    node_cnt = np.diff(gstarts)
    Np = int(_ceil(max(int(node_cnt.max()), 1), P) * P)
    assert 2 * Np <= 32767, f"Np={Np} too large for int16 gather ranges"
    NB = Np // P
    RSZ = 2 * Np

    core_of = batch // GPC
    pid = core_of * Np + (np.arange(N) - node_start[core_of])
    local_graph = batch - core_of * GPC

    deg = np.bincount(dst, weights=ew, minlength=N) + 1.0
    dinv = 1.0 / np.sqrt(deg)
    norm_e = dinv[src] * ew * dinv[dst]
    rvec = np.bincount(dst, weights=norm_e, minlength=N) + dinv * dinv

    es = np.concatenate([src, np.arange(N)])
    ed = np.concatenate([dst, np.arange(N)])
    en = np.concatenate([norm_e, dinv * dinv])

    e_core = core_of[ed]
    e_block = (pid[ed] % Np) // P
    e_dl = pid[ed] % P
    e_spid = pid[es]
    is_local = core_of[es] == e_core
    # range 0: local (idx into the core's own [Np] slice);
    # ranges 1..4: remote (idx into the full [8*Np] source, 2*Np rows each)
    NR5 = NRANGE + 1
    e_rr = np.where(is_local, 0, 1 + e_spid // RSZ)
    e_i16 = np.where(is_local, e_spid % Np, e_spid % RSZ)

    key = ((e_core * NB + e_block) * NR5 + e_rr).astype(np.int64)
    cnt = np.bincount(key, minlength=NCORES * NB * NR5).reshape(
        NCORES, NB, NR5
    )
    K = _ceil(cnt.max(axis=0), P)  # [NB, NR5]

    NG = _ceil(NB, GBLK)
    tile_of_br = np.zeros((NB, NR5), np.int64)
    chunks_by_group = [[] for _ in range(NG)]
    t = 0
    for g in range(NG):
        blks = range(g * GBLK, min((g + 1) * GBLK, NB))
        for r in range(NR5):
            t0 = t
            for b in blks:
                tile_of_br[b, r] = t
                t += int(K[b, r])
            if t > t0:
                chunks_by_group[g].append((t0, t - t0, r))
    ntiles = t

    order = np.lexsort((e_rr, e_block, e_core))
    k_sorted = key[order]
    excl = np.concatenate(
        ([0], np.cumsum(np.bincount(key, minlength=NCORES * NB * NR5)))
    )
    pos_in_bucket = np.arange(len(order)) - excl[k_sorted]
    slot_sorted = tile_of_br[e_block[order], e_rr[order]] * P + pos_in_bucket

    WWIN = _ceil(GPC, P)

    S_all, idx_all, G_all = [], [], []
    rrow = np.zeros((NCORES, Np), np.float32)
    for c in range(NCORES):
        S = np.zeros((ntiles, P, P), np.float32)
        idx_lin = np.zeros(ntiles * P, np.int16)
        m = e_core[order] == c
        sl = slot_sorted[m]
        S[sl // P, sl % P, e_dl[order][m]] = en[order][m]
        idx_lin[sl] = e_i16[order][m].astype(np.int16)
        S_all.append(
            np.ascontiguousarray(S.transpose(1, 0, 2))
            .reshape(P, ntiles * P).astype(BF)
        )
        packed = np.zeros((16, ntiles * 8), np.int16)
        for g in range(NG):
            for (t0, nt, _r) in chunks_by_group[g]:
                seg = idx_lin[t0 * P : (t0 + nt) * P]
                packed[:, t0 * 8 : (t0 + nt) * 8] = seg.reshape(-1, 16).T
        idx_all.append(np.tile(packed, (8, 1)))

        ncnt = int(node_cnt[c])
        rrow[c, :ncnt] = rvec[node_start[c] : node_start[c] + ncnt]

        G = np.zeros((NB, WWIN, P, P), np.float32)
        lg = np.full(Np, -1, np.int64)
        lg[:ncnt] = local_graph[node_start[c] : node_start[c] + ncnt]
        pp_ = np.arange(Np)
        v = lg >= 0
        w = lg[v] // P
        G[pp_[v] // P, w, pp_[v] % P, lg[v] - w * P] = 1.0
        G_all.append(
            np.ascontiguousarray(G.transpose(2, 0, 1, 3))
            .reshape(P, NB * WWIN * P).astype(BF)
        )

    x0p = np.zeros((NCORES * Np, P), np.float32)
    x0p[pid, : x.shape[1]] = np.asarray(x, np.float32)
    x0p = x0p.astype(BF)
    x0loc = [x0p[c * Np : (c + 1) * Np] for c in range(NCORES)]

    meta = dict(
        N=N, B=B, GPC=GPC, Np=Np, NB=NB, NG=NG, RSZ=RSZ, WWIN=WWIN,
        ntiles=ntiles, K=K, chunks_by_group=chunks_by_group,
        tile_of_br=tile_of_br, node_start=node_start, node_cnt=node_cnt,
    )
    per_core = [
        dict(S=S_all[c], gidx=idx_all[c], x0loc=x0loc[c],
             rrow=rrow[c].astype(BF)[None, :], G=G_all[c])
        for c in range(NCORES)
    ]
    return meta, per_core, x0p


def _fold_weights(inp):
    f = lambda k: np.asarray(inp[k], np.float64)
    A, Bb = [], []
    for i in range(1, 6):
        a = f("g%d" % i) / np.sqrt(f("v%d" % i) + 1e-5)
        A.append(a)
        Bb.append(f("be%d" % i) - f("m%d" % i) * a)

    def pack(W):
        din, dout = W.shape
        nch = _ceil(din, P)
        Wp = np.zeros((nch * P, dout))
        Wp[:din] = W
        return (
            np.ascontiguousarray(Wp.reshape(nch, P, dout).transpose(1, 0, 2))
            .reshape(P, nch * dout).astype(BF)
        )

    o = {}
    o["W1"] = pack(f("W1"))
    o["c1"] = f("b1")[None, :].astype(BF)
    o["W2"] = pack(A[0][:, None] * f("W2"))
    o["c2"] = (Bb[0] @ f("W2"))[None, :].astype(BF)
    o["b2"] = f("b2")[None, :].astype(BF)
    o["W3"] = pack(A[1][:, None] * f("W3"))
    o["c3"] = (Bb[1] @ f("W3"))[None, :].astype(BF)
    o["b3"] = f("b3")[None, :].astype(BF)
    o["W4"] = pack(A[2][:, None] * f("W4"))
    o["c4r"] = (Bb[2] @ f("W4"))[None, :].astype(BF)
    o["c4"] = f("b4")[None, :].astype(BF)
    o["W5"] = pack(A[3][:, None] * f("W5"))
    o["c5r"] = (Bb[3] @ f("W5"))[None, :].astype(BF)
    o["c5"] = f("b5")[None, :].astype(BF)
    wg = A[4] * f("Wg")[:, 0]
    o["wgrep"] = np.tile(wg[None, :], (P, 1)).astype(BF)
    o["bgrep"] = np.full(
        (P, 1), float(Bb[4] @ f("Wg")[:, 0] + f("bg")[0]), np.float32
    )
    o["Wf1"] = pack(A[4][:, None] * f("Wf1"))
    o["cf1"] = (f("bf1") + Bb[4] @ f("Wf1"))[None, :].astype(BF)
    o["Wf2"] = pack(f("Wf2"))
    o["cf2"] = f("bf2")[None, :].astype(BF)
    o["Wf3"] = pack(f("Wf3"))
    o["cf3"] = f("bf3")[None, :].astype(BF)
    return o


WSHAPES = [
    ("W1", [P, 512]), ("c1", [1, 512]),
    ("W2", [P, 4 * 256]), ("c2", [1, 256]), ("b2", [1, 256]),
    ("W3", [P, 2 * 128]), ("c3", [1, 128]), ("b3", [1, 128]),
    ("W4", [P, 256]), ("c4r", [1, 256]), ("c4", [1, 256]),
    ("W5", [P, 2 * 512]), ("c5r", [1, 512]), ("c5", [1, 512]),
    ("wgrep", [P, 512]), ("Wf1", [P, 4 * 256]), ("cf1", [1, 256]),
    ("Wf2", [P, 2 * 128]), ("cf2", [1, 128]),
    ("Wf3", [P, 1]), ("cf3", [1, 1]),
]


# ------------------------------------------------------------- device build

def build_program(meta):
    Np, NB, NG, RSZ = meta["Np"], meta["NB"], meta["NG"], meta["RSZ"]
    ntiles, K = meta["ntiles"], meta["K"]
    chunks_by_group = meta["chunks_by_group"]
    tile_of_br = meta["tile_of_br"]
    WWIN = meta["WWIN"]
    NPT = NCORES * Np

    nc = bacc.Bacc(None)
    dp = nc.declare_dram_parameter
    x0_ext = dp("x0", [NPT, P], bf16, isOutput=False)
    x0loc_ext = dp("x0loc", [Np, P], bf16, isOutput=False)
    S_ext = dp("S", [P, ntiles * P], bf16, isOutput=False)
    gidx_ext = dp("gidx", [P, ntiles * 8], i16, isOutput=False)
    rrow_ext = dp("rrow", [1, Np], bf16, isOutput=False)
    G_ext = dp("G", [P, NB * WWIN * P], bf16, isOutput=False)
    wext = {n: dp(n, sh, bf16, isOutput=False) for n, sh in WSHAPES}
    bgrep_ext = dp("bgrep", [P, 1], f32, isOutput=False)
    out_ext = dp("out", [WWIN * P, 1], f32, isOutput=True)

    u_dram = {
        1: nc.dram_tensor("u1", [Np, 512], bf16),
        2: nc.dram_tensor("u2", [Np, 256], bf16),
        3: nc.dram_tensor("u3", [Np, 128], bf16),
        4: nc.dram_tensor("u4", [Np, 256], bf16),
        5: nc.dram_tensor("u5", [Np, 512], bf16),
    }
    h_slice = {
        2: nc.dram_tensor("h2s", [Np, 256], bf16),
        3: nc.dram_tensor("h3s", [Np, 128], bf16),
    }
    full = {
        "h2": nc.dram_tensor("h2f", [NPT, 256], bf16, addr_space="Shared"),
        "h3": nc.dram_tensor("h3f", [NPT, 128], bf16, addr_space="Shared"),
        "u3": nc.dram_tensor("u3f", [NPT, 128], bf16, addr_space="Shared"),
        "u4": nc.dram_tensor("u4f", [NPT, 256], bf16, addr_space="Shared"),
    }
    RG = [list(range(NCORES))]

    with tile.TileContext(nc) as tc:
        with (
            tc.tile_pool(name="persist", bufs=1) as pp,
            tc.tile_pool(name="sb", bufs=2) as sb,
            tc.tile_pool(name="sb3", bufs=3) as sb3,
            tc.tile_pool(name="ps", bufs=2, space="PSUM") as ps,
            tc.tile_pool(name="ps_acc", bufs=2, space="PSUM") as ps_acc,
            tc.tile_pool(name="pool_ps", bufs=1, space="PSUM") as pool_ps,
        ):
            gidx_sb = pp.tile([P, ntiles * 8], i16)
            nc.sync.dma_start(out=gidx_sb[:], in_=gidx_ext[:, :])
            wsb = {}
            for n, sh in WSHAPES:
                wsb[n] = pp.tile(sh, bf16, tag="w_" + n, name="w_" + n)
                nc.sync.dma_start(out=wsb[n][:], in_=wext[n][:, :])
            bgrep_sb = pp.tile([P, 1], f32)
            nc.sync.dma_start(out=bgrep_sb[:], in_=bgrep_ext[:, :])
            ident = pp.tile([P, P], bf16)
            make_identity(nc, ident[:])
            ones_row = pp.tile([1, P], bf16)
            nc.vector.memset(ones_row[:], 1.0)
            z512 = pp.tile([1, 512], bf16)
            nc.vector.memset(z512[:], 0.0)
            eps_col = pp.tile([P, 1], f32)
            nc.vector.memset(eps_col[:], 1e-20)

            def gather_group(g, src_full, src_loc, w):
                chs = chunks_by_group[g]
                if not chs:
                    return None
                g_t0 = chs[0][0]
                g_nt = sum(nt for (_t0, nt, _r) in chs)
                gall = sb3.tile([P, g_nt, w], bf16, tag=f"gatw{w}",
                                name=f"gat_{g}")
                for (t0, nt, r) in chs:
                    off = t0 - g_t0
                    src = (src_loc[0:Np, :w] if r == 0
                           else src_full[(r - 1) * RSZ : r * RSZ, :w])
                    nc.gpsimd.dma_gather(
                        out_ap=gall[:, off : off + nt, :],
                        in_ap=src,
                        idxs_ap=gidx_sb[:, t0 * 8 : (t0 + nt) * 8],
                        num_idxs=nt * P,
                        num_idxs_reg=nt * P,
                        elem_size=w,
                    )
                s_grp = sb3.tile([P, g_nt * P], bf16, tag="sgrp",
                                 name=f"sgrp_{g}")
                nc.sync.dma_start(
                    out=s_grp[:], in_=S_ext[:, g_t0 * P : (g_t0 + g_nt) * P]
                )
                return (gall, s_grp, g_t0)

            def seg_agg(b, gts, w, bias_row):
                acc = ps_acc.tile([P, w], f32, tag="agg")
                gall, s_grp, g_t0 = gts
                first = True
                for r in range(NRANGE + 1):
                    kk = int(K[b, r])
                    for k in range(kk):
                        tg = int(tile_of_br[b, r]) + k
                        nc.tensor.matmul(
                            acc[:],
                            lhsT=s_grp[:, (tg - g_t0) * P : (tg - g_t0 + 1) * P],
                            rhs=gall[:, tg - g_t0, :],
                            start=first, stop=False,
                        )
                        first = False
                nc.tensor.matmul(
                    acc[:], lhsT=ones_row[:, :], rhs=bias_row,
                    start=first, stop=True,
                )
                return acc

            def transpose_chunks(src_sb, w):
                outs = []
                for ci in range(w // P):
                    pt = ps.tile([P, P], bf16, tag="trps")
                    nc.tensor.transpose(
                        out=pt[:], in_=src_sb[:, ci * P : (ci + 1) * P],
                        identity=ident[:],
                    )
                    st = sb.tile([P, P], bf16, tag="trsb")
                    nc.any.tensor_copy(out=st[:], in_=pt[:])
                    outs.append(st)
                return outs

            def main_matmul(lhsTs, Wn, dout, extra):
                ph = ps.tile([P, dout], f32, tag="h")
                for ci, lt in enumerate(lhsTs):
                    nc.tensor.matmul(
                        ph[:, :dout], lhsT=lt[:],
                        rhs=wsb[Wn][:, ci * dout : (ci + 1) * dout],
                        start=(ci == 0), stop=False,
                    )
                for j, (lrow, rr_) in enumerate(extra):
                    nc.tensor.matmul(
                        ph[:, :dout], lhsT=lrow, rhs=rr_,
                        start=False, stop=(j == len(extra) - 1),
                    )
                return ph

            def lrelu(psum, w, tag):
                u = sb.tile([P, w], bf16, tag=f"u{tag}")
                nc.scalar.activation(
                    out=u[:], in_=psum[:, :w],
                    func=mybir.ActivationFunctionType.Lrelu, alpha=0.01,
                )
                return u

            def load_rrow(b):
                rt = sb.tile([1, P], bf16, tag="rrow")
                nc.sync.dma_start(
                    out=rt[:], in_=rrow_ext[0:1, b * P : (b + 1) * P]
                )
                return rt

            def conv_pre(src_full, src_loc, w, Wn, dout, crn, cn, udst, tag):
                """pre-aggregate layer: agg(src) @ W (+ r x crn + 1 x cn)."""
                for g in range(min(NG, MAXG)):
                    gts = gather_group(g, src_full, src_loc, w)
                    for b in range(g * GBLK, min((g + 1) * GBLK, NB)):
                        acc = seg_agg(b, gts, w, z512[:, :w])
                        agg_sb = sb.tile([P, w], bf16, tag="aggsb")
                        nc.any.tensor_copy(out=agg_sb[:], in_=acc[:, :w])
                        lhsTs = transpose_chunks(agg_sb, w)
                        extra = []
                        if crn is not None:
                            extra.append(
                                (load_rrow(b)[:, :], wsb[crn][:, :dout])
                            )
                        extra.append((ones_row[:, :], wsb[cn][:, :dout]))
                        ph = main_matmul(lhsTs, Wn, dout, extra)
                        u = lrelu(ph, dout, tag)
                        nc.sync.dma_start(
                            out=udst[b * P : (b + 1) * P, :], in_=u[:]
                        )

            def conv_postA(usrc, w_in, Wn, dout, cn, hdst, tag):
                """h = u @ W + 1 x cn, store slice for AllGather."""
                for b in range(NB):
                    ub = sb.tile([P, w_in], bf16, tag=f"uld{tag}")
                    nc.sync.dma_start(
                        out=ub[:], in_=usrc[b * P : (b + 1) * P, :]
                    )
                    lhsTs = transpose_chunks(ub, w_in)
                    ph = main_matmul(
                        lhsTs, Wn, dout, [(ones_row[:, :], wsb[cn][:, :dout])]
                    )
                    hb = sb.tile([P, dout], bf16, tag=f"hst{tag}")
                    nc.any.tensor_copy(out=hb[:], in_=ph[:, :dout])
                    nc.sync.dma_start(
                        out=hdst[b * P : (b + 1) * P, :], in_=hb[:]
                    )

            def conv_postB(src_full, src_loc, w, bn, udst, tag):
                """agg(h_full) + bias -> lrelu -> u slice."""
                for g in range(NG):
                    gts = gather_group(g, src_full, src_loc, w)
                    for b in range(g * GBLK, min((g + 1) * GBLK, NB)):
                        acc = seg_agg(b, gts, w, wsb[bn][:, :w])
                        u = lrelu(acc, w, tag)
                        nc.sync.dma_start(
                            out=udst[b * P : (b + 1) * P, :], in_=u[:]
                        )

            def allgather(src, dst):
                nc.gpsimd.collective_compute(
                    "AllGather", mybir.AluOpType.bypass,
                    replica_groups=RG, ins=[src[:]], outs=[dst[:]],
                )

            # ----------------- conv stack -----------------
            conv_pre(x0_ext, x0loc_ext, 128, "W1", 512, None, "c1", u_dram[1], "L1")
            if PHASES >= 2:
                conv_postA(u_dram[1], 512, "W2", 256, "c2", h_slice[2], "L2")
                allgather(h_slice[2], full["h2"])
                conv_postB(full["h2"], h_slice[2], 256, "b2", u_dram[2], "L2")
            if PHASES >= 3:
                conv_postA(u_dram[2], 256, "W3", 128, "c3", h_slice[3], "L3")
                allgather(h_slice[3], full["h3"])
                conv_postB(full["h3"], h_slice[3], 128, "b3", u_dram[3], "L3")
            if PHASES >= 4:
                allgather(u_dram[3], full["u3"])
                conv_pre(full["u3"], u_dram[3], 128, "W4", 256, "c4r", "c4", u_dram[4], "L4")
                allgather(u_dram[4], full["u4"])
                conv_pre(full["u4"], u_dram[4], 256, "W5", 512, "c5r", "c5", u_dram[5], "L5")
            # ----------------- attention pooling -----------------
            def pooling():
                for w in range(WWIN):
                    pw = pool_ps.tile([P, 512], f32, tag="pw", name=f"pw{w}")
                    pe = pool_ps.tile([P, 1], f32, tag="pe", name=f"pe{w}")
                    nc.tensor.matmul(pw[:], lhsT=z512[:, :P], rhs=z512[:, :512],
                                     start=True, stop=False)
                    nc.tensor.matmul(pe[:], lhsT=z512[:, :P], rhs=z512[:, :1],
                                     start=True, stop=False)
                    for b in range(NB):
                        ub = sb.tile([P, 512], bf16, tag="u5ld")
                        nc.sync.dma_start(
                            out=ub[:], in_=u_dram[5][b * P : (b + 1) * P, :]
                        )
                        gm = sb.tile([P, 512], f32, tag="gatem")
                        nc.vector.tensor_tensor(
                            out=gm[:], in0=ub[:], in1=wsb["wgrep"][:, :],
                            op=mybir.AluOpType.mult,
                        )
                        gate = sb.tile([P, 1], f32, tag="gate")
                        nc.vector.reduce_sum(
                            out=gate[:], in_=gm[:], axis=mybir.AxisListType.X
                        )
                        e = sb.tile([P, 1], f32, tag="ecol")
                        nc.scalar.activation(
                            out=e[:], in_=gate[:],
                            func=mybir.ActivationFunctionType.Exp,
                            bias=bgrep_sb[:, :], scale=1.0,
                        )
                        e_bf = sb.tile([P, 1], bf16, tag="ebf")
                        nc.any.tensor_copy(out=e_bf[:], in_=e[:])
                        rhs512 = sb.tile([P, 512], bf16, tag="rhs512")
                        nc.vector.tensor_scalar_mul(
                            out=rhs512[:], in0=ub[:], scalar1=e[:, 0:1]
                        )
                        Gt = sb.tile([P, P], bf16, tag="Gt")
                        nc.sync.dma_start(
                            out=Gt[:],
                            in_=G_ext[:, (b * WWIN + w) * P : (b * WWIN + w + 1) * P],
                        )
                        nc.tensor.matmul(
                            pw[:], lhsT=Gt[:, :], rhs=rhs512[:],
                            start=False, stop=False,
                        )
                        nc.tensor.matmul(
                            pe[:], lhsT=Gt[:, :], rhs=e_bf[:],
                            start=False, stop=False,
                        )
                    nc.tensor.matmul(pw[:], lhsT=z512[:, :P], rhs=z512[:, :512],
                                     start=False, stop=True)
                    nc.tensor.matmul(pe[:], lhsT=z512[:, :P], rhs=z512[:, :1],
                                     start=False, stop=True)

                    pooled = sb.tile([P, 512], f32, tag="pooled")
                    nc.any.tensor_copy(out=pooled[:], in_=pw[:])
                    se = sb.tile([P, 1], f32, tag="se")
                    nc.vector.tensor_tensor(
                        out=se[:], in0=pe[:], in1=eps_col[:],
                        op=mybir.AluOpType.max,
                    )
                    si = sb.tile([P, 1], f32, tag="si")
                    nc.vector.reciprocal(out=si[:], in_=se[:])
                    fcin = sb.tile([P, 512], bf16, tag="fcin")
                    nc.vector.tensor_scalar_mul(
                        out=fcin[:], in0=pooled[:], scalar1=si[:, 0:1]
                    )
                    l1 = main_matmul(
                        transpose_chunks(fcin, 512), "Wf1", 256,
                        [(ones_row[:, :], wsb["cf1"][:, :256])],
                    )
                    h1 = lrelu(l1, 256, "fc1")
                    l2 = main_matmul(
                        transpose_chunks(h1, 256), "Wf2", 128,
                        [(ones_row[:, :], wsb["cf2"][:, :128])],
                    )
                    h2 = lrelu(l2, 128, "fc2")
                    l3 = main_matmul(
                        transpose_chunks(h2, 128), "Wf3", 1,
                        [(ones_row[:, :], wsb["cf3"][:, :1])],
                    )
                    oc = sb.tile([P, 1], f32, tag="oc")
                    nc.any.tensor_copy(out=oc[:], in_=l3[:, :1])
                    nc.sync.dma_start(
                        out=out_ext[w * P : (w + 1) * P, :], in_=oc[:]
                    )

            if PHASES >= 5:
                pooling()

    nc.finalize()
    return nc


# ----------------------------------------------------------------- frontend

_CACHE = {}


def _prepare(inputs, B):
    x = np.asarray(inputs["x"], np.float32)
    ei = np.asarray(inputs["edge_index"], np.int64)
    ea = np.asarray(inputs["edge_attr"], np.float32)
    bt = np.asarray(inputs["batch"], np.int64)
    key = hash((x.shape, ei.tobytes(), bt.tobytes(), B))
    if key not in _CACHE:
        meta, per_core, x0p = _preprocess(x, ei, ea, bt, B)
        nc = build_program(meta)
        _CACHE.clear()
        _CACHE[key] = (meta, per_core, x0p, nc)
    return _CACHE[key]


def _in_maps(meta, per_core, x0p, wf):
    maps = []
    for c in range(NCORES):
        m = dict(x0=x0p, bgrep=wf["bgrep"], **{
            n: wf[n] for n, _ in WSHAPES
        })
        m["S"] = per_core[c]["S"]
        m["x0loc"] = per_core[c]["x0loc"]
        m["gidx"] = per_core[c]["gidx"]
        m["rrow"] = per_core[c]["rrow"]
        m["G"] = per_core[c]["G"]
        maps.append(m)
    return maps


def _assemble(meta, results, inputs, B):
    GPC, WWIN = meta["GPC"], meta["WWIN"]
    out = np.empty(B, np.float32)
    for c in range(NCORES):
        out[c * GPC : (c + 1) * GPC] = results[c]["out"][:GPC, 0]
    # empty graphs: pooled == 0 exactly in the reference
    cnt = np.bincount(np.asarray(inputs["batch"], np.int64), minlength=B)
    if (cnt == 0).any():
        Wf1, bf1 = np.asarray(inputs["Wf1"]), np.asarray(inputs["bf1"])
        Wf2, bf2 = np.asarray(inputs["Wf2"]), np.asarray(inputs["bf2"])
        Wf3, bf3 = np.asarray(inputs["Wf3"]), np.asarray(inputs["bf3"])
        lr = lambda z: np.where(z >= 0, z, 0.01 * z)
        h = lr(np.zeros(Wf1.shape[0]) @ Wf1 + bf1)
        h = lr(h @ Wf2 + bf2)
        out[cnt == 0] = float(h @ Wf3 + bf3)
    return out


def kernel(_B=B_DEFAULT, **inputs):
    meta, per_core, x0p, nc = _prepare(inputs, _B)
    wf = _fold_weights(inputs)
    maps = _in_maps(meta, per_core, x0p, wf)
    res = run_bass_kernel_spmd(nc, maps, core_ids=list(range(NCORES)))
    return _assemble(meta, res.results, inputs, _B)



# revision 17
# speedup vs baseline: 1.5477x; 1.5477x over previous
"""GCNNet forward on 8 Trainium2 NeuronCores (Bass/Tile SPMD), v2.

Strategy
--------
- Nodes partitioned graph-aligned across 8 cores (B/8 graphs per core).
- Edge aggregation (GCN symmetric norm) via one-hot matmuls in SWAPPED
  orientation: accT[feat, dst] += gathered[slot, feat]^T-free @ S[slot, dst].
  The PSUM result is directly the lhsT for the next dense matmul, so the
  conv stack needs NO PE transposes.
- Self-loops are NOT gathered: per dst block, lhsT = contiguous local
  feature block, rhs = diag(dinv^2) tile (D). Removes ~30% of gather rows.
- L1 gathers x at its true width 64.
- u1/u2 are stored transposed ([feat, node]) so the following matmul-first
  layers (L2/L3 postA) read lhsT chunks directly. h2/h3/u3/u4 stay
  node-major for AllGather + gather.
- BatchNorm affine + biases folded into weights host-side (rank-1 rows
  with rrow[n] = row-sum of norm incl self).
- Cross-core exchange: 4 bf16 AllGathers (h2, h3, u3, u4).
- Attention pooling: single pass over u5; per-graph one-hot matmuls into
  per-window PSUM accumulators, only (block, window) pairs that are
  nonzero on some core. Softmax without max-subtraction; host fixes
  empty graphs.
"""
import os
import sys

for _p in ("/opt/trn_rl_repo", "/root/.axon_site/_ro/trn_rl_repo"):
    if os.path.isdir(_p) and _p not in sys.path:
        sys.path.insert(0, _p)

import numpy as np
import ml_dtypes

import concourse.bass as bass
import concourse.bacc as bacc
import concourse.mybir as mybir
import concourse.tile as tile
from concourse.bass_utils import run_bass_kernel_spmd
from concourse.masks import make_identity

P = 128
NCORES = 8
NRANGE = 4
GBLK = 1  # blocks per gather group (HW dma_gather breaks with large chunks)

bf16 = mybir.dt.float16  # working dtype (fp16: 10-bit mantissa, safe ranges)
f32 = mybir.dt.float32
i16 = mybir.dt.int16
BF = np.float16

B_DEFAULT = 2048
PHASES = 5  # debug: how many phases of the program to emit
REPS = 1  # timing: repeat the whole body REPS times inside the program
MAXG = 10**9


def set_f32_debug():
    global bf16, BF
    bf16 = mybir.dt.float32
    BF = np.float32


def set_f16():
    global bf16, BF
    bf16 = mybir.dt.float16
    BF = np.float16


def _ceil(a, b):
    return -(-a // b)


# ----------------------------------------------------------------- host prep

def _preprocess(x, edge_index, edge_attr, batch, B):
    N = x.shape[0]
    GPC = B // NCORES
    src = np.asarray(edge_index[0], np.int64)
    dst = np.asarray(edge_index[1], np.int64)
    ew = np.asarray(edge_attr, np.float64)
    batch = np.asarray(batch, np.int64)

    gstarts = np.searchsorted(batch, np.arange(0, B + 1, GPC))
    node_start = gstarts[:-1]
    node_cnt = np.diff(gstarts)
    Np = int(_ceil(max(int(node_cnt.max()), 1), P) * P)
    assert 2 * Np <= 32767, f"Np={Np} too large for int16 gather ranges"
    NB = Np // P
    RSZ = 2 * Np

    core_of = batch // GPC
    pid = core_of * Np + (np.arange(N) - node_start[core_of])
    local_graph = batch - core_of * GPC

    deg = np.bincount(dst, weights=ew, minlength=N) + 1.0
    dinv = 1.0 / np.sqrt(deg)
    dinv2 = dinv * dinv
    norm_e = dinv[src] * ew * dinv[dst]
    rvec = np.bincount(dst, weights=norm_e, minlength=N) + dinv2

    # edges only (self-loops handled via D diag tiles)
    es, ed, en = src, dst, norm_e

    e_core = core_of[ed]
    e_block = (pid[ed] % Np) // P
    e_dl = pid[ed] % P
    e_spid = pid[es]
    is_local = core_of[es] == e_core
    NR5 = NRANGE + 1
    e_rr = np.where(is_local, 0, 1 + e_spid // RSZ)
    e_i16 = np.where(is_local, e_spid % Np, e_spid % RSZ)

    key = ((e_core * NB + e_block) * NR5 + e_rr).astype(np.int64)
    cnt = np.bincount(key, minlength=NCORES * NB * NR5).reshape(
        NCORES, NB, NR5
    )
    K = _ceil(cnt.max(axis=0), P)  # [NB, NR5]

    NG = _ceil(NB, GBLK)
    tile_of_br = np.zeros((NB, NR5), np.int64)
    chunks_by_group = [[] for _ in range(NG)]
    t = 0
    for g in range(NG):
        blks = range(g * GBLK, min((g + 1) * GBLK, NB))
        for r in range(NR5):
            t0 = t
            for b in blks:
                tile_of_br[b, r] = t
                t += int(K[b, r])
            if t > t0:
                chunks_by_group[g].append((t0, t - t0, r))
    ntiles = t

    order = np.lexsort((e_rr, e_block, e_core))
    k_sorted = key[order]
    excl = np.concatenate(
        ([0], np.cumsum(np.bincount(key, minlength=NCORES * NB * NR5)))
    )
    pos_in_bucket = np.arange(len(order)) - excl[k_sorted]
    slot_sorted = tile_of_br[e_block[order], e_rr[order]] * P + pos_in_bucket

    WWIN = _ceil(GPC, P)

    # pooling (block, window) pairs: union over cores of nonzero G tiles
    lg_all = []
    pair_set = set()
    for c in range(NCORES):
        lg = np.full(Np, -1, np.int64)
        ncnt = int(node_cnt[c])
        lg[:ncnt] = local_graph[node_start[c] : node_start[c] + ncnt]
        lg_all.append(lg)
        for b in range(NB):
            seg = lg[b * P : (b + 1) * P]
            for w in np.unique(seg[seg >= 0] // P):
                pair_set.add((b, int(w)))
    pairs = sorted(pair_set)
    npairs = len(pairs)
    pairs_by_block = {}
    for j, (b, w) in enumerate(pairs):
        pairs_by_block.setdefault(b, []).append((j, w))

    S_all, idx_all, G_all, D_all = [], [], [], []
    rrow = np.zeros((NCORES, Np), np.float32)
    for c in range(NCORES):
        S = np.zeros((ntiles, P, P), np.float32)
        idx_lin = np.zeros(ntiles * P, np.int16)
        m = e_core[order] == c
        sl = slot_sorted[m]
        S[sl // P, sl % P, e_dl[order][m]] = en[order][m]
        idx_lin[sl] = e_i16[order][m].astype(np.int16)
        S_all.append(
            np.ascontiguousarray(S.transpose(1, 0, 2))
            .reshape(P, ntiles * P).astype(BF)
        )
        packed = np.zeros((16, ntiles * 8), np.int16)
        for g in range(NG):
            for (t0, nt, _r) in chunks_by_group[g]:
                seg = idx_lin[t0 * P : (t0 + nt) * P]
                packed[:, t0 * 8 : (t0 + nt) * 8] = seg.reshape(-1, 16).T
        idx_all.append(np.tile(packed, (8, 1)))

        ncnt = int(node_cnt[c])
        rrow[c, :ncnt] = rvec[node_start[c] : node_start[c] + ncnt]

        D = np.zeros((NB, P, P), np.float32)
        dloc = dinv2[node_start[c] : node_start[c] + ncnt]
        nn = np.arange(ncnt)
        D[nn // P, nn % P, nn % P] = dloc
        D_all.append(
            np.ascontiguousarray(D.transpose(1, 0, 2))
            .reshape(P, NB * P).astype(BF)
        )

        G = np.zeros((npairs, P, P), np.float32)
        lg = lg_all[c]
        for j, (b, w) in enumerate(pairs):
            seg = lg[b * P : (b + 1) * P]
            v = (seg >= 0) & (seg // P == w)
            G[j, np.nonzero(v)[0], seg[v] - w * P] = 1.0
        G_all.append(
            np.ascontiguousarray(G.transpose(1, 0, 2))
            .reshape(P, npairs * P).astype(BF)
        )

    IN = x.shape[1]
    # dma_gather requires elem_size to be a multiple of 256B -> pad x to 128
    x0p = np.zeros((NCORES * Np, P), np.float32)
    x0p[pid, :IN] = np.asarray(x, np.float32)
    x0p = x0p.astype(BF)
    x0loc = [x0p[c * Np : (c + 1) * Np] for c in range(NCORES)]

    meta = dict(
        N=N, B=B, GPC=GPC, Np=Np, NB=NB, NG=NG, RSZ=RSZ, WWIN=WWIN,
        ntiles=ntiles, K=K, chunks_by_group=chunks_by_group,
        tile_of_br=tile_of_br, node_start=node_start, node_cnt=node_cnt,
        IN=IN, npairs=npairs, pairs=pairs, pairs_by_block=pairs_by_block,
    )
    per_core = [
        dict(S=S_all[c], gidx=idx_all[c], x0loc=x0loc[c],
             rrow=rrow[c].astype(BF)[None, :], G=G_all[c], D=D_all[c])
        for c in range(NCORES)
    ]
    return meta, per_core, x0p


def _fold_weights(inp):
    f = lambda k: np.asarray(inp[k], np.float64)
    A, Bb = [], []
    for i in range(1, 6):
        a = f("g%d" % i) / np.sqrt(f("v%d" % i) + 1e-5)
        A.append(a)
        Bb.append(f("be%d" % i) - f("m%d" % i) * a)

    def pack(W):
        din, dout = W.shape
        nch = _ceil(din, P)
        Wp = np.zeros((nch * P, dout))
        Wp[:din] = W
        return (
            np.ascontiguousarray(Wp.reshape(nch, P, dout).transpose(1, 0, 2))
            .reshape(P, nch * dout).astype(BF)
        )

    o = {}
    o["W1"] = pack(f("W1"))
    o["c1"] = f("b1")[None, :].astype(BF)
    o["W2"] = pack(A[0][:, None] * f("W2"))
    o["c2"] = (Bb[0] @ f("W2"))[None, :].astype(BF)
    o["b2"] = f("b2")[None, :].astype(BF)
    o["W3"] = pack(A[1][:, None] * f("W3"))
    o["c3"] = (Bb[1] @ f("W3"))[None, :].astype(BF)
    o["b3"] = f("b3")[None, :].astype(BF)
    o["W4"] = pack(A[2][:, None] * f("W4"))
    o["c4r"] = (Bb[2] @ f("W4"))[None, :].astype(BF)
    o["c4"] = f("b4")[None, :].astype(BF)
    o["W5"] = pack(A[3][:, None] * f("W5"))
    o["c5r"] = (Bb[3] @ f("W5"))[None, :].astype(BF)
    o["c5"] = f("b5")[None, :].astype(BF)
    wg = A[4] * f("Wg")[:, 0]
    o["wgrep"] = np.tile(wg[None, :], (P, 1)).astype(BF)
    o["bgrep"] = np.full(
        (P, 1), float(Bb[4] @ f("Wg")[:, 0] + f("bg")[0]), np.float32
    )
    o["Wf1"] = pack(A[4][:, None] * f("Wf1"))
    o["cf1"] = (f("bf1") + Bb[4] @ f("Wf1"))[None, :].astype(BF)
    o["Wf2"] = pack(f("Wf2"))
    o["cf2"] = f("bf2")[None, :].astype(BF)
    o["Wf3"] = pack(f("Wf3"))
    o["cf3"] = f("bf3")[None, :].astype(BF)
    return o


WSHAPES = [
    ("W1", [P, 512]), ("c1", [1, 512]),
    ("W2", [P, 4 * 256]), ("c2", [1, 256]), ("b2", [1, 256]),
    ("W3", [P, 2 * 128]), ("c3", [1, 128]), ("b3", [1, 128]),
    ("W4", [P, 256]), ("c4r", [1, 256]), ("c4", [1, 256]),
    ("W5", [P, 2 * 512]), ("c5r", [1, 512]), ("c5", [1, 512]),
    ("wgrep", [P, 512]), ("Wf1", [P, 4 * 256]), ("cf1", [1, 256]),
    ("Wf2", [P, 2 * 128]), ("cf2", [1, 128]),
    ("Wf3", [P, 1]), ("cf3", [1, 1]),
]


# ------------------------------------------------------------- device build

def build_program(meta):
    Np, NB, NG, RSZ = meta["Np"], meta["NB"], meta["NG"], meta["RSZ"]
    ntiles, K = meta["ntiles"], meta["K"]
    chunks_by_group = meta["chunks_by_group"]
    tile_of_br = meta["tile_of_br"]
    WWIN, IN = meta["WWIN"], meta["IN"]
    npairs, pairs_by_block = meta["npairs"], meta["pairs_by_block"]
    NPT = NCORES * Np

    nc = bacc.Bacc(None)
    dp = nc.declare_dram_parameter
    x0_ext = dp("x0", [NPT, P], bf16, isOutput=False)
    x0loc_ext = dp("x0loc", [Np, P], bf16, isOutput=False)
    S_ext = dp("S", [P, ntiles * P], bf16, isOutput=False)
    gidx_ext = dp("gidx", [P, ntiles * 8], i16, isOutput=False)
    rrow_ext = dp("rrow", [1, Np], bf16, isOutput=False)
    G_ext = dp("G", [P, npairs * P], bf16, isOutput=False)
    D_ext = dp("D", [P, NB * P], bf16, isOutput=False)
    wext = {n: dp(n, sh, bf16, isOutput=False) for n, sh in WSHAPES}
    bgrep_ext = dp("bgrep", [P, 1], f32, isOutput=False)
    out_ext = dp("out", [WWIN * P, 1], f32, isOutput=True)

    u1T = nc.dram_tensor("u1T", [4 * P, Np], bf16)
    u2T = nc.dram_tensor("u2T", [2 * P, Np], bf16)
    h2s = nc.dram_tensor("h2s", [Np, 256], bf16)
    h3s = nc.dram_tensor("h3s", [Np, 128], bf16)
    u3 = nc.dram_tensor("u3", [Np, 128], bf16)
    u4 = nc.dram_tensor("u4", [Np, 256], bf16)
    u5 = nc.dram_tensor("u5", [Np, 512], bf16)
    full = {
        "h2": nc.dram_tensor("h2f", [NPT, 256], bf16, addr_space="Shared"),
        "h3": nc.dram_tensor("h3f", [NPT, 128], bf16, addr_space="Shared"),
        "u3": nc.dram_tensor("u3f", [NPT, 128], bf16, addr_space="Shared"),
        "u4": nc.dram_tensor("u4f", [NPT, 256], bf16, addr_space="Shared"),
    }
    RG = [list(range(NCORES))]

    with tile.TileContext(nc) as tc:
        with (
            tc.tile_pool(name="persist", bufs=1) as pp,
            tc.tile_pool(name="sb", bufs=2) as sb,
            tc.tile_pool(name="sb3", bufs=3) as sb3,
            tc.tile_pool(name="ps", bufs=2, space="PSUM") as ps,
            tc.tile_pool(name="ps_acc", bufs=2, space="PSUM") as ps_acc,
        ):
            gidx_sb = pp.tile([P, ntiles * 8], i16)
            nc.sync.dma_start(out=gidx_sb[:], in_=gidx_ext[:, :])
            wsb = {}
            for n, sh in WSHAPES:
                wsb[n] = pp.tile(sh, bf16, tag="w_" + n, name="w_" + n)
                nc.sync.dma_start(out=wsb[n][:], in_=wext[n][:, :])
            bgrep_sb = pp.tile([P, 1], f32)
            nc.sync.dma_start(out=bgrep_sb[:], in_=bgrep_ext[:, :])
            D_sb = pp.tile([P, NB * P], bf16, name="D_sb")
            nc.sync.dma_start(out=D_sb[:], in_=D_ext[:, :])
            G_sb = pp.tile([P, npairs * P], bf16, name="G_sb")
            nc.sync.dma_start(out=G_sb[:], in_=G_ext[:, :])
            rrow_sb = pp.tile([1, Np], bf16, name="rrow_sb")
            nc.sync.dma_start(out=rrow_sb[:], in_=rrow_ext[:, :])
            ident = pp.tile([P, P], bf16)
            make_identity(nc, ident[:])
            ones_row = pp.tile([1, P], bf16)
            nc.vector.memset(ones_row[:], 1.0)
            eps_col = pp.tile([P, 1], f32)
            nc.vector.memset(eps_col[:], 1e-20)

            def gather_group(g, src_full, src_loc, w):
                chs = chunks_by_group[g]
                if not chs:
                    return None
                g_t0 = chs[0][0]
                g_nt = sum(nt for (_t0, nt, _r) in chs)
                gall = sb3.tile([P, g_nt, w], bf16, tag=f"gatw{w}",
                                name=f"gat_{g}")
                for (t0, nt, r) in chs:
                    off = t0 - g_t0
                    src = (src_loc[0:Np, :w] if r == 0
                           else src_full[(r - 1) * RSZ : r * RSZ, :w])
                    nc.gpsimd.dma_gather(
                        out_ap=gall[:, off : off + nt, :],
                        in_ap=src,
                        idxs_ap=gidx_sb[:, t0 * 8 : (t0 + nt) * 8],
                        num_idxs=nt * P,
                        num_idxs_reg=nt * P,
                        elem_size=w,
                    )
                s_grp = sb3.tile([P, g_nt * P], bf16, tag="sgrp",
                                 name=f"sgrp_{g}")
                nc.sync.dma_start(
                    out=s_grp[:], in_=S_ext[:, g_t0 * P : (g_t0 + g_nt) * P]
                )
                return (gall, s_grp, g_t0)

            def load_self4(src_loc, g4, w, tag):
                """contiguous rows for blocks 4*g4 .. 4*g4+3 -> [P, 4, w]."""
                nb4 = min(4, NB - g4 * 4)
                t = sb.tile([P, 4, w], bf16, tag=tag)
                nc.sync.dma_start(
                    out=t[:, :nb4, :],
                    in_=src_loc[g4 * 4 * P : (g4 * 4 + nb4) * P, :w].rearrange(
                        "(a p) w -> p a w", p=P
                    ),
                )
                return t

            def seg_aggT(b, gts, w, self_sb, bias_row):
                """swapped-orientation aggregation: accT[feat, dst].

                Returns list of PSUM tiles, one per 128-chunk of w. w may be
                smaller than the gathered width (trailing cols ignored).
                self_sb: [P, >=w] node-major contiguous self rows for block b.
                bias_row: [1, w] row to add per-dst (or None).
                """
                gall, s_grp, g_t0 = gts
                nchunk = _ceil(w, P)
                accs = []
                for c in range(nchunk):
                    cw = min(P, w - c * P)
                    acc = ps_acc.tile([P, 512], f32, tag=f"agg{'AB'[c]}",
                                      name=f"agg{c}")
                    first = True
                    for r in range(NRANGE + 1):
                        kk = int(K[b, r])
                        for k in range(kk):
                            tg = int(tile_of_br[b, r]) + k
                            nc.tensor.matmul(
                                acc[:cw, :P],
                                lhsT=gall[:, tg - g_t0, c * P : c * P + cw],
                                rhs=s_grp[:, (tg - g_t0) * P : (tg - g_t0 + 1) * P],
                                start=first, stop=False,
                            )
                            first = False
                    last = bias_row is None
                    nc.tensor.matmul(
                        acc[:cw, :P],
                        lhsT=self_sb[:, c * P : c * P + cw],
                        rhs=D_sb[:, b * P : (b + 1) * P],
                        start=first, stop=last,
                    )
                    if bias_row is not None:
                        nc.tensor.matmul(
                            acc[:cw, :P],
                            lhsT=bias_row[:, c * P : c * P + cw],
                            rhs=ones_row[:, :],
                            start=False, stop=True,
                        )
                    accs.append((acc, cw))
                return accs

            def accs_to_sbuf(accs, tag):
                outs = []
                for i, (acc, cw) in enumerate(accs):
                    st = sb.tile([P, P], bf16, tag=f"{tag}{i}")
                    nc.vector.tensor_copy(out=st[:cw, :], in_=acc[:cw, :P])
                    outs.append((st, cw))
                return outs

            def lrelu(psum, w, tag):
                u = sb.tile([P, w], bf16, tag=f"u{tag}")
                nc.scalar.activation(
                    out=u[:], in_=psum[:, :w],
                    func=mybir.ActivationFunctionType.Lrelu, alpha=0.01,
                )
                return u

            # -------------------------------------------------- L1 conv_pre
            def conv1():
                for g4 in range(_ceil(NB, 4)):
                    selfs = load_self4(x0loc_ext, g4, P, "self1")
                    stages = [
                        sb.tile([P, 4 * P], bf16, tag=f"u1st{oc}",
                                name=f"u1st{oc}")
                        for oc in range(4)
                    ]
                    for a in range(4):
                        b = g4 * 4 + a
                        if b >= NB:
                            break
                        gts = gather_group(b, x0_ext, x0loc_ext, P)
                        accs = seg_aggT(b, gts, IN, selfs[:, a, :], None)
                        aggT = accs_to_sbuf(accs, "aggT1")[0]
                        st0, cw = aggT
                        for oc in range(4):
                            phT = ps.tile([P, 512], f32, tag="h",
                                          name="phT")
                            nc.tensor.matmul(
                                phT[:, :P],
                                lhsT=wsb["W1"][:cw, oc * P : (oc + 1) * P],
                                rhs=st0[:cw, :],
                                start=True, stop=False,
                            )
                            nc.tensor.matmul(
                                phT[:, :P],
                                lhsT=wsb["c1"][:, oc * P : (oc + 1) * P],
                                rhs=ones_row[:, :],
                                start=False, stop=True,
                            )
                            nc.scalar.activation(
                                out=stages[oc][:, a * P : (a + 1) * P],
                                in_=phT[:, :P],
                                func=mybir.ActivationFunctionType.Lrelu,
                                alpha=0.01,
                            )
                    nb4 = min(4, NB - g4 * 4)
                    for oc in range(4):
                        nc.sync.dma_start(
                            out=u1T[oc * P : (oc + 1) * P,
                                    g4 * 4 * P : (g4 * 4 + nb4) * P],
                            in_=stages[oc][:, : nb4 * P],
                        )

            # ---------------------------------------- postA: h = u @ W + c
            def conv_postA(uT_src, nch, Wn, dout, cn, hdst, tag):
                for g4 in range(_ceil(NB, 4)):
                    nb4g = min(4, NB - g4 * 4)
                    ins = []
                    for ic in range(nch):
                        t = sb.tile([P, 4 * P], bf16, tag=f"pA{tag}{ic}")
                        nc.sync.dma_start(
                            out=t[:, : nb4g * P],
                            in_=uT_src[ic * P : (ic + 1) * P,
                                       g4 * 4 * P : (g4 * 4 + nb4g) * P],
                        )
                        ins.append(t)
                    for a in range(4):
                        b = g4 * 4 + a
                        if b >= NB:
                            break
                        ph = ps.tile([P, 512], f32, tag="h", name=f"h{tag}")
                        for ic in range(nch):
                            nc.tensor.matmul(
                                ph[:, :dout],
                                lhsT=ins[ic][:, a * P : (a + 1) * P],
                                rhs=wsb[Wn][:, ic * dout : (ic + 1) * dout],
                                start=(ic == 0), stop=False,
                            )
                        nc.tensor.matmul(
                            ph[:, :dout], lhsT=ones_row[:, :],
                            rhs=wsb[cn][:, :dout],
                            start=False, stop=True,
                        )
                        hb = sb.tile([P, dout], bf16, tag=f"hst{tag}")
                        nc.vector.tensor_copy(out=hb[:], in_=ph[:, :dout])
                        nc.sync.dma_start(
                            out=hdst[b * P : (b + 1) * P, :], in_=hb[:]
                        )

            # ------------------- postB: agg(h) + b -> lrelu -> uT (swapped)
            def conv_postB_T(src_full, src_loc, w, bn, uT_dst, tag):
                nchunk = w // P
                for g4 in range(_ceil(NB, 4)):
                    selfs = load_self4(src_loc, g4, w, f"selfB{tag}")
                    stages = [
                        sb.tile([P, 4 * P], bf16, tag=f"uB{tag}{c}",
                                name=f"uB{tag}{c}")
                        for c in range(nchunk)
                    ]
                    for a in range(4):
                        b = g4 * 4 + a
                        if b >= NB:
                            break
                        gts = gather_group(b, src_full, src_loc, w)
                        accs = seg_aggT(b, gts, w, selfs[:, a, :],
                                        wsb[bn][:, :w])
                        for c, (acc, cw) in enumerate(accs):
                            nc.scalar.activation(
                                out=stages[c][:, a * P : (a + 1) * P],
                                in_=acc[:cw, :P],
                                func=mybir.ActivationFunctionType.Lrelu,
                                alpha=0.01,
                            )
                    nb4 = min(4, NB - g4 * 4)
                    for c in range(nchunk):
                        nc.sync.dma_start(
                            out=uT_dst[c * P : (c + 1) * P,
                                       g4 * 4 * P : (g4 * 4 + nb4) * P],
                            in_=stages[c][:, : nb4 * P],
                        )

            # ------- postB normal orientation: agg(h) + b -> lrelu -> u [Np,w]
            def conv_postB_N(src_full, src_loc, w, bn, u_dst, tag):
                for g4 in range(_ceil(NB, 4)):
                    selfs = load_self4(src_loc, g4, w, f"selfN{tag}")
                    stage = sb.tile([P, 4, w], bf16, tag=f"uN{tag}")
                    for a in range(4):
                        b = g4 * 4 + a
                        if b >= NB:
                            break
                        gts = gather_group(b, src_full, src_loc, w)
                        gall, s_grp, g_t0 = gts
                        acc = ps_acc.tile([P, 512], f32, tag="aggA",
                                          name="aggN")
                        first = True
                        for r in range(NRANGE + 1):
                            kk = int(K[b, r])
                            for k in range(kk):
                                tg = int(tile_of_br[b, r]) + k
                                nc.tensor.matmul(
                                    acc[:, :w],
                                    lhsT=s_grp[:, (tg - g_t0) * P : (tg - g_t0 + 1) * P],
                                    rhs=gall[:, tg - g_t0, :],
                                    start=first, stop=False,
                                )
                                first = False
                        nc.tensor.matmul(
                            acc[:, :w], lhsT=D_sb[:, b * P : (b + 1) * P],
                            rhs=selfs[:, a, :], start=first, stop=False,
                        )
                        nc.tensor.matmul(
                            acc[:, :w], lhsT=ones_row[:, :], rhs=wsb[bn][:, :w],
                            start=False, stop=True,
                        )
                        nc.scalar.activation(
                            out=stage[:, a, :], in_=acc[:, :w],
                            func=mybir.ActivationFunctionType.Lrelu,
                            alpha=0.01,
                        )
                    nb4 = min(4, NB - g4 * 4)
                    nc.sync.dma_start(
                        out=u_dst[g4 * 4 * P : (g4 * 4 + nb4) * P, :]
                        .rearrange("(a p) w -> p a w", p=P),
                        in_=stage[:, :nb4, :],
                    )

            # -------- conv_pre (L4/L5): agg(u) swapped, then main matmul
            def conv_pre(src_full, src_loc, w, Wn, dout, crn, cn, u_dst, tag):
                for g4 in range(_ceil(NB, 4)):
                    selfs = load_self4(src_loc, g4, w, f"selfP{tag}")
                    stage = sb.tile([P, 4, dout], bf16, tag=f"uP{tag}")
                    for a in range(4):
                        b = g4 * 4 + a
                        if b >= NB:
                            break
                        gts = gather_group(b, src_full, src_loc, w)
                        accs = seg_aggT(b, gts, w, selfs[:, a, :], None)
                        aggTs = accs_to_sbuf(accs, f"agg{tag}")
                        ph = ps.tile([P, 512], f32, tag="h", name=f"hP{tag}")
                        for c, (st, cw) in enumerate(aggTs):
                            nc.tensor.matmul(
                                ph[:, :dout],
                                lhsT=st[:cw, :],
                                rhs=wsb[Wn][:cw, c * dout : (c + 1) * dout],
                                start=(c == 0), stop=False,
                            )
                        nc.tensor.matmul(
                            ph[:, :dout],
                            lhsT=rrow_sb[:, b * P : (b + 1) * P],
                            rhs=wsb[crn][:, :dout],
                            start=False, stop=False,
                        )
                        nc.tensor.matmul(
                            ph[:, :dout], lhsT=ones_row[:, :],
                            rhs=wsb[cn][:, :dout],
                            start=False, stop=True,
                        )
                        nc.scalar.activation(
                            out=stage[:, a, :], in_=ph[:, :dout],
                            func=mybir.ActivationFunctionType.Lrelu,
                            alpha=0.01,
                        )
                    nb4 = min(4, NB - g4 * 4)
                    nc.sync.dma_start(
                        out=u_dst[g4 * 4 * P : (g4 * 4 + nb4) * P, :]
                        .rearrange("(a p) w -> p a w", p=P),
                        in_=stage[:, :nb4, :],
                    )

            def allgather(src, dst):
                nc.gpsimd.collective_compute(
                    "AllGather", mybir.AluOpType.bypass,
                    replica_groups=RG, ins=[src[:]], outs=[dst[:]],
                )

            def transpose_chunks(src_sb, w):
                outs = []
                for ci in range(w // P):
                    pt = ps.tile([P, P], bf16, tag="trps")
                    nc.tensor.transpose(
                        out=pt[:], in_=src_sb[:, ci * P : (ci + 1) * P],
                        identity=ident[:],
                    )
                    st = sb.tile([P, P], bf16, tag="trsb")
                    nc.vector.tensor_copy(out=st[:], in_=pt[:])
                    outs.append(st)
                return outs

            def main_matmul(lhsTs, Wn, dout, extra):
                ph = ps.tile([P, 512], f32, tag="h", name="hfc")
                for ci, lt in enumerate(lhsTs):
                    nc.tensor.matmul(
                        ph[:, :dout], lhsT=lt[:],
                        rhs=wsb[Wn][:, ci * dout : (ci + 1) * dout],
                        start=(ci == 0), stop=False,
                    )
                for j, (lrow, rr_) in enumerate(extra):
                    nc.tensor.matmul(
                        ph[:, :dout], lhsT=lrow, rhs=rr_,
                        start=False, stop=(j == len(extra) - 1),
                    )
                return ph

            # ----------------------------------------------- pooling (1 pass)
            def pooling(rep):
                pw = [ps_acc.tile([P, 512], f32, tag="aggA",
                                  name=f"pw{w}_{rep}") for w in range(WWIN)]
                pe = [ps_acc.tile([P, 512], f32, tag="aggB",
                                  name=f"pe{w}_{rep}") for w in range(WWIN)]
                z1 = sb.tile([1, 512], bf16, tag="zrow")
                nc.vector.memset(z1[:], 0.0)
                for w in range(WWIN):
                    nc.tensor.matmul(pw[w][:, :512], lhsT=z1[:, :P],
                                     rhs=z1[:, :512], start=True, stop=False)
                    nc.tensor.matmul(pe[w][:, :1], lhsT=z1[:, :P],
                                     rhs=z1[:, :1], start=True, stop=False)
                for g4 in range(_ceil(NB, 4)):
                    u5g = sb3.tile([P, 4, 512], bf16, tag="u5ld")
                    nb4 = min(4, NB - g4 * 4)
                    nc.sync.dma_start(
                        out=u5g[:, :nb4, :],
                        in_=u5[g4 * 4 * P : (g4 * 4 + nb4) * P, :].rearrange(
                            "(a p) w -> p a w", p=P
                        ),
                    )
                    for a in range(nb4):
                        b = g4 * 4 + a
                        plist = pairs_by_block.get(b, [])
                        if not plist:
                            continue
                        ub = u5g[:, a, :]
                        gm = sb.tile([P, 512], f32, tag="gatem")
                        nc.vector.tensor_tensor(
                            out=gm[:], in0=ub, in1=wsb["wgrep"][:, :],
                            op=mybir.AluOpType.mult,
                        )
                        gate = sb.tile([P, 1], f32, tag="gate")
                        nc.vector.reduce_sum(
                            out=gate[:], in_=gm[:], axis=mybir.AxisListType.X
                        )
                        e = sb.tile([P, 1], f32, tag="ecol")
                        nc.scalar.activation(
                            out=e[:], in_=gate[:],
                            func=mybir.ActivationFunctionType.Exp,
                            bias=bgrep_sb[:, :], scale=1.0,
                        )
                        e_bf = sb.tile([P, 1], bf16, tag="ebf")
                        nc.vector.tensor_copy(out=e_bf[:], in_=e[:])
                        rhs512 = sb.tile([P, 512], bf16, tag="rhs512")
                        nc.vector.tensor_scalar_mul(
                            out=rhs512[:], in0=ub, scalar1=e[:, 0:1]
                        )
                        for (j, w) in plist:
                            nc.tensor.matmul(
                                pw[w][:, :512],
                                lhsT=G_sb[:, j * P : (j + 1) * P],
                                rhs=rhs512[:], start=False, stop=False,
                            )
                            nc.tensor.matmul(
                                pe[w][:, :1],
                                lhsT=G_sb[:, j * P : (j + 1) * P],
                                rhs=e_bf[:], start=False, stop=False,
                            )
                for w in range(WWIN):
                    nc.tensor.matmul(pw[w][:, :512], lhsT=z1[:, :P],
                                     rhs=z1[:, :512], start=False, stop=True)
                    nc.tensor.matmul(pe[w][:, :1], lhsT=z1[:, :P],
                                     rhs=z1[:, :1], start=False, stop=True)

                    pooled = sb.tile([P, 512], f32, tag="pooled")
                    nc.vector.tensor_copy(out=pooled[:], in_=pw[w][:, :512])
                    se = sb.tile([P, 1], f32, tag="se")
                    nc.vector.tensor_tensor(
                        out=se[:], in0=pe[w][:, :1], in1=eps_col[:],
                        op=mybir.AluOpType.max,
                    )
                    si = sb.tile([P, 1], f32, tag="si")
                    nc.vector.reciprocal(out=si[:], in_=se[:])
                    fcin = sb.tile([P, 512], bf16, tag="fcin")
                    nc.vector.tensor_scalar_mul(
                        out=fcin[:], in0=pooled[:], scalar1=si[:, 0:1]
                    )
                    l1 = main_matmul(
                        transpose_chunks(fcin, 512), "Wf1", 256,
                        [(ones_row[:, :], wsb["cf1"][:, :256])],
                    )
                    h1 = lrelu(l1, 256, "fc1")
                    l2 = main_matmul(
                        transpose_chunks(h1, 256), "Wf2", 128,
                        [(ones_row[:, :], wsb["cf2"][:, :128])],
                    )
                    h2 = lrelu(l2, 128, "fc2")
                    l3 = main_matmul(
                        transpose_chunks(h2, 128), "Wf3", 1,
                        [(ones_row[:, :], wsb["cf3"][:, :1])],
                    )
                    oc = sb.tile([P, 1], f32, tag="oc")
                    nc.vector.tensor_copy(out=oc[:], in_=l3[:, :1])
                    nc.sync.dma_start(
                        out=out_ext[w * P : (w + 1) * P, :], in_=oc[:]
                    )

            # ----------------- the program -----------------
            for rep in range(REPS):
                conv1()
                if PHASES >= 2:
                    conv_postA(u1T, 4, "W2", 256, "c2", h2s, "L2")
                    allgather(h2s, full["h2"])
                    conv_postB_T(full["h2"], h2s, 256, "b2", u2T, "L2")
                if PHASES >= 3:
                    conv_postA(u2T, 2, "W3", 128, "c3", h3s, "L3")
                    allgather(h3s, full["h3"])
                    conv_postB_N(full["h3"], h3s, 128, "b3", u3, "L3")
                if PHASES >= 4:
                    allgather(u3, full["u3"])
                    conv_pre(full["u3"], u3, 128, "W4", 256, "c4r", "c4",
                             u4, "L4")
                    allgather(u4, full["u4"])
                    conv_pre(full["u4"], u4, 256, "W5", 512, "c5r", "c5",
                             u5, "L5")
                if PHASES >= 5:
                    pooling(rep)

    nc.finalize()
    return nc


# ----------------------------------------------------------------- frontend

_CACHE = {}


def _prepare(inputs, B):
    x = np.asarray(inputs["x"], np.float32)
    ei = np.asarray(inputs["edge_index"], np.int64)
    ea = np.asarray(inputs["edge_attr"], np.float32)
    bt = np.asarray(inputs["batch"], np.int64)
    key = hash((x.shape, ei.tobytes(), bt.tobytes(), B))
    if key not in _CACHE:
        meta, per_core, x0p = _preprocess(x, ei, ea, bt, B)
        nc = build_program(meta)
        _CACHE.clear()
        _CACHE[key] = (meta, per_core, x0p, nc)
    return _CACHE[key]


def _in_maps(meta, per_core, x0p, wf):
    maps = []
    for c in range(NCORES):
        m = dict(x0=x0p, bgrep=wf["bgrep"], **{
            n: wf[n] for n, _ in WSHAPES
        })
        m["S"] = per_core[c]["S"]
        m["x0loc"] = per_core[c]["x0loc"]
        m["gidx"] = per_core[c]["gidx"]
        m["rrow"] = per_core[c]["rrow"]
        m["G"] = per_core[c]["G"]
        m["D"] = per_core[c]["D"]
        maps.append(m)
    return maps


def _assemble(meta, results, inputs, B):
    GPC, WWIN = meta["GPC"], meta["WWIN"]
    out = np.empty(B, np.float32)
    for c in range(NCORES):
        out[c * GPC : (c + 1) * GPC] = results[c]["out"][:GPC, 0]
    cnt = np.bincount(np.asarray(inputs["batch"], np.int64), minlength=B)
    if (cnt == 0).any():
        Wf1, bf1 = np.asarray(inputs["Wf1"]), np.asarray(inputs["bf1"])
        Wf2, bf2 = np.asarray(inputs["Wf2"]), np.asarray(inputs["bf2"])
        Wf3, bf3 = np.asarray(inputs["Wf3"]), np.asarray(inputs["bf3"])
        lr = lambda z: np.where(z >= 0, z, 0.01 * z)
        h = lr(np.zeros(Wf1.shape[0]) @ Wf1 + bf1)
        h = lr(h @ Wf2 + bf2)
        out[cnt == 0] = float(h @ Wf3 + bf3)
    return out


def kernel(_B=B_DEFAULT, **inputs):
    meta, per_core, x0p, nc = _prepare(inputs, _B)
    wf = _fold_weights(inputs)
    maps = _in_maps(meta, per_core, x0p, wf)
    res = run_bass_kernel_spmd(nc, maps, core_ids=list(range(NCORES)))
    return _assemble(meta, res.results, inputs, _B)


# revision 23
# speedup vs baseline: 1.6142x; 1.0429x over previous
"""GCNNet forward on 8 Trainium2 NeuronCores (Bass/Tile SPMD), v2.

Strategy
--------
- Nodes partitioned graph-aligned across 8 cores (B/8 graphs per core).
- Edge aggregation (GCN symmetric norm) via one-hot matmuls in SWAPPED
  orientation: accT[feat, dst] += gathered[slot, feat]^T-free @ S[slot, dst].
  The PSUM result is directly the lhsT for the next dense matmul, so the
  conv stack needs NO PE transposes.
- Self-loops are NOT gathered: per dst block, lhsT = contiguous local
  feature block, rhs = diag(dinv^2) tile (D). Removes ~30% of gather rows.
- L1 gathers x at its true width 64.
- u1/u2 are stored transposed ([feat, node]) so the following matmul-first
  layers (L2/L3 postA) read lhsT chunks directly. h2/h3/u3/u4 stay
  node-major for AllGather + gather.
- BatchNorm affine + biases folded into weights host-side (rank-1 rows
  with rrow[n] = row-sum of norm incl self).
- Cross-core exchange: 4 bf16 AllGathers (h2, h3, u3, u4).
- Attention pooling: single pass over u5; per-graph one-hot matmuls into
  per-window PSUM accumulators, only (block, window) pairs that are
  nonzero on some core. Softmax without max-subtraction; host fixes
  empty graphs.
"""
import os
import sys

for _p in ("/opt/trn_rl_repo", "/root/.axon_site/_ro/trn_rl_repo"):
    if os.path.isdir(_p) and _p not in sys.path:
        sys.path.insert(0, _p)

import numpy as np
import ml_dtypes

import concourse.bass as bass
import concourse.bacc as bacc
import concourse.mybir as mybir
import concourse.tile as tile
from concourse.bass_utils import run_bass_kernel_spmd
from concourse.masks import make_identity

P = 128
NCORES = 8
NRANGE = 4
GBLK = 1  # blocks per gather group (HW dma_gather breaks with large chunks)

bf16 = mybir.dt.float16  # working dtype (fp16: 10-bit mantissa, safe ranges)
f32 = mybir.dt.float32
i16 = mybir.dt.int16
BF = np.float16

B_DEFAULT = 2048
PHASES = 5  # debug: how many phases of the program to emit
REPS = 1  # timing: repeat the whole body REPS times inside the program
MAXG = 10**9


def set_f32_debug():
    global bf16, BF
    bf16 = mybir.dt.float32
    BF = np.float32


def set_f16():
    global bf16, BF
    bf16 = mybir.dt.float16
    BF = np.float16


def _ceil(a, b):
    return -(-a // b)


# ----------------------------------------------------------------- host prep

def _preprocess(x, edge_index, edge_attr, batch, B):
    N = x.shape[0]
    GPC = B // NCORES
    src = np.asarray(edge_index[0], np.int64)
    dst = np.asarray(edge_index[1], np.int64)
    ew = np.asarray(edge_attr, np.float64)
    batch = np.asarray(batch, np.int64)

    gstarts = np.searchsorted(batch, np.arange(0, B + 1, GPC))
    node_start = gstarts[:-1]
    node_cnt = np.diff(gstarts)
    Np = int(_ceil(max(int(node_cnt.max()), 1), P) * P)
    assert 2 * Np <= 32767, f"Np={Np} too large for int16 gather ranges"
    NB = Np // P
    RSZ = 2 * Np

    core_of = batch // GPC
    pid = core_of * Np + (np.arange(N) - node_start[core_of])
    local_graph = batch - core_of * GPC

    deg = np.bincount(dst, weights=ew, minlength=N) + 1.0
    dinv = 1.0 / np.sqrt(deg)
    dinv2 = dinv * dinv
    norm_e = dinv[src] * ew * dinv[dst]
    rvec = np.bincount(dst, weights=norm_e, minlength=N) + dinv2

    # edges only (self-loops handled via D diag tiles)
    es, ed, en = src, dst, norm_e

    e_core = core_of[ed]
    e_block = (pid[ed] % Np) // P
    e_dl = pid[ed] % P

    WWIN = _ceil(GPC, P)

    # ---- selective exchange (AllToAll): per (sender, receiver) row lists
    s_core_e = core_of[es]
    l_src = pid[es] % Np  # sender-local row id
    sendlists = [[None] * NCORES for _ in range(NCORES)]
    maxlen = 1
    for s in range(NCORES):
        for r in range(NCORES):
            mm = (s_core_e == s) & (e_core == r)
            u = np.unique(l_src[mm])
            sendlists[s][r] = u
            maxlen = max(maxlen, len(u))
    CH = _ceil(maxlen, 64) * 64  # 2*CH must be a multiple of 128
    NSEND = NCORES * CH
    NSEND = _ceil(NSEND, P) * P
    assert NSEND <= 32767, f"NSEND={NSEND} exceeds int16 gather range"
    NST = NSEND // P  # send tiles

    # pool row for each edge: s*CH + position in sendlists[s][r]
    pool_row = np.empty(len(es), np.int64)
    for s in range(NCORES):
        for r in range(NCORES):
            mm = (s_core_e == s) & (e_core == r)
            if mm.any():
                pool_row[mm] = s * CH + np.searchsorted(
                    sendlists[s][r], l_src[mm]
                )

    # single-range bucketing for the 4 exchanged layers
    key2 = (e_core * NB + e_block).astype(np.int64)
    cnt2 = np.bincount(key2, minlength=NCORES * NB).reshape(NCORES, NB)
    K2 = _ceil(cnt2.max(axis=0), P)  # [NB]
    tile_of_b2 = np.concatenate(([0], np.cumsum(K2)))[:-1]
    ntilesE = int(K2.sum())
    order2 = np.lexsort((e_block, e_core))
    excl2 = np.concatenate(
        ([0], np.cumsum(np.bincount(key2, minlength=NCORES * NB)))
    )
    pos2 = np.arange(len(order2)) - excl2[key2[order2]]
    slot2 = tile_of_b2[e_block[order2]] * P + pos2

    # pooling (block, window) pairs: union over cores of nonzero G tiles
    lg_all = []
    pair_set = set()
    for c in range(NCORES):
        lg = np.full(Np, -1, np.int64)
        ncnt = int(node_cnt[c])
        lg[:ncnt] = local_graph[node_start[c] : node_start[c] + ncnt]
        lg_all.append(lg)
        for b in range(NB):
            seg = lg[b * P : (b + 1) * P]
            for w in np.unique(seg[seg >= 0] // P):
                pair_set.add((b, int(w)))
    pairs = sorted(pair_set)
    npairs = len(pairs)
    pairs_by_block = {}
    for j, (b, w) in enumerate(pairs):
        pairs_by_block.setdefault(b, []).append((j, w))

    G_all, D_all = [], []
    S2_all, idx2_all, sidx_all, sidx0_all = [], [], [], []
    rrow = np.zeros((NCORES, Np), np.float32)
    for c in range(NCORES):
        # exchanged-layer S2/gidx2 (single range into the A2A pool)
        S2 = np.zeros((ntilesE, P, P), np.float32)
        idx2_lin = np.zeros(ntilesE * P, np.int16)
        m2 = e_core[order2] == c
        sl2 = slot2[m2]
        S2[sl2 // P, sl2 % P, e_dl[order2][m2]] = en[order2][m2]
        idx2_lin[sl2] = pool_row[order2][m2].astype(np.int16)
        S2_all.append(
            np.ascontiguousarray(S2.transpose(1, 0, 2))
            .reshape(P, ntilesE * P).astype(BF)
        )
        p2 = idx2_lin.reshape(-1, 16).T
        idx2_all.append(np.tile(p2, (8, 1)))

        sidx_lin = np.zeros(NSEND, np.int16)
        for r in range(NCORES):
            L = sendlists[c][r]
            sidx_lin[r * CH : r * CH + len(L)] = L.astype(np.int16)
        ps_ = sidx_lin.reshape(-1, 16).T
        sidx_all.append(np.tile(ps_, (8, 1)))

        # pool0 build indices: row s*CH+i <- x0[s*Np + sendlists[s][c][i]],
        # gathered per range r'=s//2 from x0[r'*RSZ:(r'+1)*RSZ]
        sidx0_lin = np.zeros(NSEND, np.int16)
        for s in range(NCORES):
            L = sendlists[s][c]
            sidx0_lin[s * CH : s * CH + len(L)] = (
                (s % 2) * Np + L
            ).astype(np.int16)
        p0_ = sidx0_lin.reshape(-1, 16).T
        sidx0_all.append(np.tile(p0_, (8, 1)))

        ncnt = int(node_cnt[c])
        rrow[c, :ncnt] = rvec[node_start[c] : node_start[c] + ncnt]

        D = np.zeros((NB, P, P), np.float32)
        dloc = dinv2[node_start[c] : node_start[c] + ncnt]
        nn = np.arange(ncnt)
        D[nn // P, nn % P, nn % P] = dloc
        D_all.append(
            np.ascontiguousarray(D.transpose(1, 0, 2))
            .reshape(P, NB * P).astype(BF)
        )

        G = np.zeros((npairs, P, P), np.float32)
        lg = lg_all[c]
        for j, (b, w) in enumerate(pairs):
            seg = lg[b * P : (b + 1) * P]
            v = (seg >= 0) & (seg // P == w)
            G[j, np.nonzero(v)[0], seg[v] - w * P] = 1.0
        G_all.append(
            np.ascontiguousarray(G.transpose(1, 0, 2))
            .reshape(P, npairs * P).astype(BF)
        )

    IN = x.shape[1]
    # dma_gather requires elem_size to be a multiple of 256B -> pad x to 128
    x0p = np.zeros((NCORES * Np, P), np.float32)
    x0p[pid, :IN] = np.asarray(x, np.float32)
    x0p = x0p.astype(BF)
    x0loc = [x0p[c * Np : (c + 1) * Np] for c in range(NCORES)]

    meta = dict(
        N=N, B=B, GPC=GPC, Np=Np, NB=NB, RSZ=RSZ, WWIN=WWIN,
        node_start=node_start, node_cnt=node_cnt,
        IN=IN, npairs=npairs, pairs=pairs, pairs_by_block=pairs_by_block,
        CH=CH, NSEND=NSEND, NST=NST, K2=K2, tile_of_b2=tile_of_b2,
        ntilesE=ntilesE,
    )
    per_core = [
        dict(x0loc=x0loc[c],
             rrow=rrow[c].astype(BF)[None, :], G=G_all[c], D=D_all[c],
             S2=S2_all[c], gidx2=idx2_all[c], sidx=sidx_all[c],
             sidx0=sidx0_all[c])
        for c in range(NCORES)
    ]
    return meta, per_core, x0p


def _fold_weights(inp):
    f = lambda k: np.asarray(inp[k], np.float64)
    A, Bb = [], []
    for i in range(1, 6):
        a = f("g%d" % i) / np.sqrt(f("v%d" % i) + 1e-5)
        A.append(a)
        Bb.append(f("be%d" % i) - f("m%d" % i) * a)

    def pack(W):
        din, dout = W.shape
        nch = _ceil(din, P)
        Wp = np.zeros((nch * P, dout))
        Wp[:din] = W
        return (
            np.ascontiguousarray(Wp.reshape(nch, P, dout).transpose(1, 0, 2))
            .reshape(P, nch * dout).astype(BF)
        )

    o = {}
    o["W1"] = pack(f("W1"))
    o["c1"] = f("b1")[None, :].astype(BF)
    o["W2"] = pack(A[0][:, None] * f("W2"))
    o["c2"] = (Bb[0] @ f("W2"))[None, :].astype(BF)
    o["b2"] = f("b2")[None, :].astype(BF)
    o["W3"] = pack(A[1][:, None] * f("W3"))
    o["c3"] = (Bb[1] @ f("W3"))[None, :].astype(BF)
    o["b3"] = f("b3")[None, :].astype(BF)
    o["W4"] = pack(A[2][:, None] * f("W4"))
    o["c4r"] = (Bb[2] @ f("W4"))[None, :].astype(BF)
    o["c4"] = f("b4")[None, :].astype(BF)
    o["W5"] = pack(A[3][:, None] * f("W5"))
    o["c5r"] = (Bb[3] @ f("W5"))[None, :].astype(BF)
    o["c5"] = f("b5")[None, :].astype(BF)
    wg = A[4] * f("Wg")[:, 0]
    o["wgrep"] = np.tile(wg[None, :], (P, 1)).astype(BF)
    o["bgrep"] = np.full(
        (P, 1), float(Bb[4] @ f("Wg")[:, 0] + f("bg")[0]), np.float32
    )
    o["Wf1"] = pack(A[4][:, None] * f("Wf1"))
    o["cf1"] = (f("bf1") + Bb[4] @ f("Wf1"))[None, :].astype(BF)
    o["Wf2"] = pack(f("Wf2"))
    o["cf2"] = f("bf2")[None, :].astype(BF)
    o["Wf3"] = pack(f("Wf3"))
    o["cf3"] = f("bf3")[None, :].astype(BF)
    return o


WSHAPES = [
    ("W1", [P, 512]), ("c1", [1, 512]),
    ("W2", [P, 4 * 256]), ("c2", [1, 256]), ("b2", [1, 256]),
    ("W3", [P, 2 * 128]), ("c3", [1, 128]), ("b3", [1, 128]),
    ("W4", [P, 256]), ("c4r", [1, 256]), ("c4", [1, 256]),
    ("W5", [P, 2 * 512]), ("c5r", [1, 512]), ("c5", [1, 512]),
    ("wgrep", [P, 512]), ("Wf1", [P, 4 * 256]), ("cf1", [1, 256]),
    ("Wf2", [P, 2 * 128]), ("cf2", [1, 128]),
    ("Wf3", [P, 1]), ("cf3", [1, 1]),
]


# ------------------------------------------------------------- device build

def build_program(meta):
    Np, NB, RSZ = meta["Np"], meta["NB"], meta["RSZ"]
    WWIN, IN = meta["WWIN"], meta["IN"]
    npairs, pairs_by_block = meta["npairs"], meta["pairs_by_block"]
    NSEND, NST = meta["NSEND"], meta["NST"]
    K2, tile_of_b2, ntilesE = meta["K2"], meta["tile_of_b2"], meta["ntilesE"]
    NPT = NCORES * Np

    nc = bacc.Bacc(None)
    dp = nc.declare_dram_parameter
    x0_ext = dp("x0", [NPT, P], bf16, isOutput=False)
    x0loc_ext = dp("x0loc", [Np, P], bf16, isOutput=False)
    S2_ext = dp("S2", [P, ntilesE * P], bf16, isOutput=False)
    gidx2_ext = dp("gidx2", [P, ntilesE * 8], i16, isOutput=False)
    sidx_ext = dp("sidx", [P, NST * 8], i16, isOutput=False)
    sidx0_ext = dp("sidx0", [P, NST * 8], i16, isOutput=False)
    rrow_ext = dp("rrow", [1, Np], bf16, isOutput=False)
    G_ext = dp("G", [P, npairs * P], bf16, isOutput=False)
    D_ext = dp("D", [P, NB * P], bf16, isOutput=False)
    wext = {n: dp(n, sh, bf16, isOutput=False) for n, sh in WSHAPES}
    bgrep_ext = dp("bgrep", [P, 1], f32, isOutput=False)
    out_ext = dp("out", [WWIN * P, 1], f32, isOutput=True)

    u1T = nc.dram_tensor("u1T", [4 * P, Np], bf16)
    u2T = nc.dram_tensor("u2T", [2 * P, Np], bf16)
    h2s = nc.dram_tensor("h2s", [Np, 256], bf16)
    h3s = nc.dram_tensor("h3s", [Np, 128], bf16)
    u3 = nc.dram_tensor("u3", [Np, 128], bf16)
    u4 = nc.dram_tensor("u4", [Np, 256], bf16)
    u5 = nc.dram_tensor("u5", [Np, 512], bf16)
    pool0 = nc.dram_tensor("pool0", [NSEND, P], bf16)
    sendA = nc.dram_tensor("sendA", [NSEND, 256], bf16)
    sendB = nc.dram_tensor("sendB", [NSEND, 128], bf16)
    pool2 = nc.dram_tensor("pool2", [NSEND, 256], bf16)
    pool3 = nc.dram_tensor("pool3", [NSEND, 128], bf16)
    poolu3 = nc.dram_tensor("poolu3", [NSEND, 128], bf16)
    poolu4 = nc.dram_tensor("poolu4", [NSEND, 256], bf16)
    RG = [list(range(NCORES))]

    with tile.TileContext(nc) as tc:
        with (
            tc.tile_pool(name="persist", bufs=1) as pp,
            tc.tile_pool(name="sb", bufs=2) as sb,
            tc.tile_pool(name="sb3", bufs=3) as sb3,
            tc.tile_pool(name="ps", bufs=2, space="PSUM") as ps,
            tc.tile_pool(name="ps_acc", bufs=2, space="PSUM") as ps_acc,
        ):
            gidx2_sb = pp.tile([P, ntilesE * 8], i16)
            nc.sync.dma_start(out=gidx2_sb[:], in_=gidx2_ext[:, :])
            sidx_sb = pp.tile([P, NST * 8], i16)
            nc.sync.dma_start(out=sidx_sb[:], in_=sidx_ext[:, :])
            sidx0_sb = pp.tile([P, NST * 8], i16)
            nc.sync.dma_start(out=sidx0_sb[:], in_=sidx0_ext[:, :])
            wsb = {}
            for n, sh in WSHAPES:
                wsb[n] = pp.tile(sh, bf16, tag="w_" + n, name="w_" + n)
                nc.sync.dma_start(out=wsb[n][:], in_=wext[n][:, :])
            bgrep_sb = pp.tile([P, 1], f32)
            nc.sync.dma_start(out=bgrep_sb[:], in_=bgrep_ext[:, :])
            D_sb = pp.tile([P, NB * P], bf16, name="D_sb")
            nc.sync.dma_start(out=D_sb[:], in_=D_ext[:, :])
            rrow_sb = pp.tile([1, Np], bf16, name="rrow_sb")
            nc.sync.dma_start(out=rrow_sb[:], in_=rrow_ext[:, :])
            ident = pp.tile([P, P], bf16)
            make_identity(nc, ident[:])
            ones_row = pp.tile([1, P], bf16)
            nc.vector.memset(ones_row[:], 1.0)
            eps_col = pp.tile([P, 1], f32)
            nc.vector.memset(eps_col[:], 1e-20)

            def pool0build():
                """local build of the L1 pool from the replicated x0:
                sender s's chunk lies in range s//2 of x0."""
                GW = 8
                ntr = NST // 4  # tiles per range (2*CH rows)
                for rp in range(4):
                    base = rp * ntr
                    for gg in range(_ceil(ntr, GW)):
                        nt = min(GW, ntr - gg * GW)
                        t = sb.tile([P, GW, P], bf16, tag="p0b",
                                    name=f"p0b_{rp}_{gg}")
                        nc.gpsimd.dma_gather(
                            out_ap=t[:, :nt, :],
                            in_ap=x0_ext[rp * RSZ : (rp + 1) * RSZ, :],
                            idxs_ap=sidx0_sb[
                                :, (base + gg * GW) * 8 : (base + gg * GW + nt) * 8
                            ],
                            num_idxs=nt * P,
                            num_idxs_reg=nt * P,
                            elem_size=P,
                        )
                        nc.sync.dma_start(
                            out=pool0[
                                (base + gg * GW) * P : (base + gg * GW + nt) * P, :
                            ].rearrange("(a p) w -> p a w", p=P),
                            in_=t[:, :nt, :],
                        )

            def gather_pool(b, pool, w):
                """exchanged layers: single-range gather from the A2A pool."""
                t0 = int(tile_of_b2[b])
                nt = int(K2[b])
                gall = sb3.tile([P, nt, w], bf16, tag=f"gpw{w}",
                                name=f"gp_{b}")
                nc.gpsimd.dma_gather(
                    out_ap=gall[:, :, :],
                    in_ap=pool[0:NSEND, :w],
                    idxs_ap=gidx2_sb[:, t0 * 8 : (t0 + nt) * 8],
                    num_idxs=nt * P,
                    num_idxs_reg=nt * P,
                    elem_size=w,
                )
                s_grp = sb3.tile([P, nt * P], bf16, tag="sgrp2",
                                 name=f"sg2_{b}")
                nc.sync.dma_start(
                    out=s_grp[:], in_=S2_ext[:, t0 * P : (t0 + nt) * P]
                )
                return (gall, s_grp, t0, [t0 + k for k in range(nt)])

            def sendbuild(src_loc, send_dram, w):
                """gather own rows every peer needs into the A2A send buf."""
                GW = 8
                for gg in range(_ceil(NST, GW)):
                    nt = min(GW, NST - gg * GW)
                    t = sb.tile([P, GW, w], bf16, tag=f"snd{w}",
                                name=f"snd{w}_{gg}")
                    nc.gpsimd.dma_gather(
                        out_ap=t[:, :nt, :],
                        in_ap=src_loc[0:Np, :w],
                        idxs_ap=sidx_sb[:, gg * GW * 8 : (gg * GW + nt) * 8],
                        num_idxs=nt * P,
                        num_idxs_reg=nt * P,
                        elem_size=w,
                    )
                    nc.sync.dma_start(
                        out=send_dram[gg * GW * P : (gg * GW + nt) * P, :]
                        .rearrange("(a p) w -> p a w", p=P),
                        in_=t[:, :nt, :],
                    )

            def a2a(src, dst):
                nc.gpsimd.collective_compute(
                    "AllToAll", mybir.AluOpType.bypass,
                    replica_groups=RG, ins=[src[:]], outs=[dst[:]],
                )

            def load_self4(src_loc, g4, w, tag):
                """contiguous rows for blocks 4*g4 .. 4*g4+3 -> [P, 4, w]."""
                nb4 = min(4, NB - g4 * 4)
                t = sb.tile([P, 4, w], bf16, tag=tag)
                nc.sync.dma_start(
                    out=t[:, :nb4, :],
                    in_=src_loc[g4 * 4 * P : (g4 * 4 + nb4) * P, :w].rearrange(
                        "(a p) w -> p a w", p=P
                    ),
                )
                return t

            def seg_aggT(b, gts, w, self_sb, bias_row):
                """swapped-orientation aggregation: accT[feat, dst].

                Returns list of PSUM tiles, one per 128-chunk of w. w may be
                smaller than the gathered width (trailing cols ignored).
                self_sb: [P, >=w] node-major contiguous self rows for block b.
                bias_row: [1, w] row to add per-dst (or None).
                """
                gall, s_grp, g_t0, tiles = gts
                nchunk = _ceil(w, P)
                accs = []
                for c in range(nchunk):
                    cw = min(P, w - c * P)
                    acc = ps_acc.tile([P, 512], f32, tag=f"agg{'AB'[c]}",
                                      name=f"agg{c}")
                    first = True
                    for tg in tiles:
                        nc.tensor.matmul(
                            acc[:cw, :P],
                            lhsT=gall[:, tg - g_t0, c * P : c * P + cw],
                            rhs=s_grp[:, (tg - g_t0) * P : (tg - g_t0 + 1) * P],
                            start=first, stop=False,
                        )
                        first = False
                    last = bias_row is None
                    nc.tensor.matmul(
                        acc[:cw, :P],
                        lhsT=self_sb[:, c * P : c * P + cw],
                        rhs=D_sb[:, b * P : (b + 1) * P],
                        start=first, stop=last,
                    )
                    if bias_row is not None:
                        nc.tensor.matmul(
                            acc[:cw, :P],
                            lhsT=bias_row[:, c * P : c * P + cw],
                            rhs=ones_row[:, :],
                            start=False, stop=True,
                        )
                    accs.append((acc, cw))
                return accs

            def accs_to_sbuf(accs, tag):
                outs = []
                for i, (acc, cw) in enumerate(accs):
                    st = sb.tile([P, P], bf16, tag=f"{tag}{i}")
                    nc.vector.tensor_copy(out=st[:cw, :], in_=acc[:cw, :P])
                    outs.append((st, cw))
                return outs

            def lrelu(psum, w, tag):
                u = sb.tile([P, w], bf16, tag=f"u{tag}")
                nc.scalar.activation(
                    out=u[:], in_=psum[:, :w],
                    func=mybir.ActivationFunctionType.Lrelu, alpha=0.01,
                )
                return u

            # -------------------------------------------------- L1 conv_pre
            def conv1():
                for g4 in range(_ceil(NB, 4)):
                    selfs = load_self4(x0loc_ext, g4, P, "self1")
                    stages = [
                        sb.tile([P, 4 * P], bf16, tag=f"u1st{oc}",
                                name=f"u1st{oc}")
                        for oc in range(4)
                    ]
                    for a in range(4):
                        b = g4 * 4 + a
                        if b >= NB:
                            break
                        gts = gather_pool(b, pool0, P)
                        accs = seg_aggT(b, gts, IN, selfs[:, a, :], None)
                        aggT = accs_to_sbuf(accs, "aggT1")[0]
                        st0, cw = aggT
                        for oc in range(4):
                            phT = ps.tile([P, 512], f32, tag="h",
                                          name="phT")
                            nc.tensor.matmul(
                                phT[:, :P],
                                lhsT=wsb["W1"][:cw, oc * P : (oc + 1) * P],
                                rhs=st0[:cw, :],
                                start=True, stop=False,
                            )
                            nc.tensor.matmul(
                                phT[:, :P],
                                lhsT=wsb["c1"][:, oc * P : (oc + 1) * P],
                                rhs=ones_row[:, :],
                                start=False, stop=True,
                            )
                            nc.scalar.activation(
                                out=stages[oc][:, a * P : (a + 1) * P],
                                in_=phT[:, :P],
                                func=mybir.ActivationFunctionType.Lrelu,
                                alpha=0.01,
                            )
                    nb4 = min(4, NB - g4 * 4)
                    for oc in range(4):
                        nc.sync.dma_start(
                            out=u1T[oc * P : (oc + 1) * P,
                                    g4 * 4 * P : (g4 * 4 + nb4) * P],
                            in_=stages[oc][:, : nb4 * P],
                        )

            # ---------------------------------------- postA: h = u @ W + c
            def conv_postA(uT_src, nch, Wn, dout, cn, hdst, tag):
                for g4 in range(_ceil(NB, 4)):
                    nb4g = min(4, NB - g4 * 4)
                    ins = []
                    for ic in range(nch):
                        t = sb.tile([P, 4 * P], bf16, tag=f"pA{tag}{ic}")
                        nc.sync.dma_start(
                            out=t[:, : nb4g * P],
                            in_=uT_src[ic * P : (ic + 1) * P,
                                       g4 * 4 * P : (g4 * 4 + nb4g) * P],
                        )
                        ins.append(t)
                    for a in range(4):
                        b = g4 * 4 + a
                        if b >= NB:
                            break
                        ph = ps.tile([P, 512], f32, tag="h", name=f"h{tag}")
                        for ic in range(nch):
                            nc.tensor.matmul(
                                ph[:, :dout],
                                lhsT=ins[ic][:, a * P : (a + 1) * P],
                                rhs=wsb[Wn][:, ic * dout : (ic + 1) * dout],
                                start=(ic == 0), stop=False,
                            )
                        nc.tensor.matmul(
                            ph[:, :dout], lhsT=ones_row[:, :],
                            rhs=wsb[cn][:, :dout],
                            start=False, stop=True,
                        )
                        hb = sb.tile([P, dout], bf16, tag=f"hst{tag}")
                        nc.vector.tensor_copy(out=hb[:], in_=ph[:, :dout])
                        nc.sync.dma_start(
                            out=hdst[b * P : (b + 1) * P, :], in_=hb[:]
                        )

            # ------------------- postB: agg(h) + b -> lrelu -> uT (swapped)
            def conv_postB_T(pool, src_loc, w, bn, uT_dst, tag):
                nchunk = w // P
                for g4 in range(_ceil(NB, 4)):
                    selfs = load_self4(src_loc, g4, w, f"selfB{tag}")
                    stages = [
                        sb.tile([P, 4 * P], bf16, tag=f"uB{tag}{c}",
                                name=f"uB{tag}{c}")
                        for c in range(nchunk)
                    ]
                    for a in range(4):
                        b = g4 * 4 + a
                        if b >= NB:
                            break
                        gts = gather_pool(b, pool, w)
                        accs = seg_aggT(b, gts, w, selfs[:, a, :],
                                        wsb[bn][:, :w])
                        for c, (acc, cw) in enumerate(accs):
                            nc.scalar.activation(
                                out=stages[c][:, a * P : (a + 1) * P],
                                in_=acc[:cw, :P],
                                func=mybir.ActivationFunctionType.Lrelu,
                                alpha=0.01,
                            )
                    nb4 = min(4, NB - g4 * 4)
                    for c in range(nchunk):
                        nc.sync.dma_start(
                            out=uT_dst[c * P : (c + 1) * P,
                                       g4 * 4 * P : (g4 * 4 + nb4) * P],
                            in_=stages[c][:, : nb4 * P],
                        )

            # ------- postB normal orientation: agg(h) + b -> lrelu -> u [Np,w]
            def conv_postB_N(pool, src_loc, w, bn, u_dst, tag):
                for g4 in range(_ceil(NB, 4)):
                    selfs = load_self4(src_loc, g4, w, f"selfN{tag}")
                    stage = sb.tile([P, 4, w], bf16, tag=f"uN{tag}")
                    for a in range(4):
                        b = g4 * 4 + a
                        if b >= NB:
                            break
                        gts = gather_pool(b, pool, w)
                        gall, s_grp, g_t0, tiles = gts
                        acc = ps_acc.tile([P, 512], f32, tag="aggA",
                                          name="aggN")
                        first = True
                        for tg in tiles:
                            nc.tensor.matmul(
                                acc[:, :w],
                                lhsT=s_grp[:, (tg - g_t0) * P : (tg - g_t0 + 1) * P],
                                rhs=gall[:, tg - g_t0, :],
                                start=first, stop=False,
                            )
                            first = False
                        nc.tensor.matmul(
                            acc[:, :w], lhsT=D_sb[:, b * P : (b + 1) * P],
                            rhs=selfs[:, a, :], start=first, stop=False,
                        )
                        nc.tensor.matmul(
                            acc[:, :w], lhsT=ones_row[:, :], rhs=wsb[bn][:, :w],
                            start=False, stop=True,
                        )
                        nc.scalar.activation(
                            out=stage[:, a, :], in_=acc[:, :w],
                            func=mybir.ActivationFunctionType.Lrelu,
                            alpha=0.01,
                        )
                    nb4 = min(4, NB - g4 * 4)
                    nc.sync.dma_start(
                        out=u_dst[g4 * 4 * P : (g4 * 4 + nb4) * P, :]
                        .rearrange("(a p) w -> p a w", p=P),
                        in_=stage[:, :nb4, :],
                    )

            # -------- conv_pre (L4/L5): agg(u) swapped, then main matmul
            def conv_pre(pool, src_loc, w, Wn, dout, crn, cn, u_dst, tag):
                for g4 in range(_ceil(NB, 4)):
                    selfs = load_self4(src_loc, g4, w, f"selfP{tag}")
                    stage = sb.tile([P, 4, dout], bf16, tag=f"uP{tag}")
                    for a in range(4):
                        b = g4 * 4 + a
                        if b >= NB:
                            break
                        gts = gather_pool(b, pool, w)
                        accs = seg_aggT(b, gts, w, selfs[:, a, :], None)
                        aggTs = accs_to_sbuf(accs, f"agg{tag}")
                        ph = ps.tile([P, 512], f32, tag="h", name=f"hP{tag}")
                        for c, (st, cw) in enumerate(aggTs):
                            nc.tensor.matmul(
                                ph[:, :dout],
                                lhsT=st[:cw, :],
                                rhs=wsb[Wn][:cw, c * dout : (c + 1) * dout],
                                start=(c == 0), stop=False,
                            )
                        nc.tensor.matmul(
                            ph[:, :dout],
                            lhsT=rrow_sb[:, b * P : (b + 1) * P],
                            rhs=wsb[crn][:, :dout],
                            start=False, stop=False,
                        )
                        nc.tensor.matmul(
                            ph[:, :dout], lhsT=ones_row[:, :],
                            rhs=wsb[cn][:, :dout],
                            start=False, stop=True,
                        )
                        nc.scalar.activation(
                            out=stage[:, a, :], in_=ph[:, :dout],
                            func=mybir.ActivationFunctionType.Lrelu,
                            alpha=0.01,
                        )
                    nb4 = min(4, NB - g4 * 4)
                    nc.sync.dma_start(
                        out=u_dst[g4 * 4 * P : (g4 * 4 + nb4) * P, :]
                        .rearrange("(a p) w -> p a w", p=P),
                        in_=stage[:, :nb4, :],
                    )

            def transpose_chunks(src_sb, w):
                outs = []
                for ci in range(w // P):
                    pt = ps.tile([P, P], bf16, tag="trps")
                    nc.tensor.transpose(
                        out=pt[:], in_=src_sb[:, ci * P : (ci + 1) * P],
                        identity=ident[:],
                    )
                    st = sb.tile([P, P], bf16, tag="trsb")
                    nc.vector.tensor_copy(out=st[:], in_=pt[:])
                    outs.append(st)
                return outs

            def main_matmul(lhsTs, Wn, dout, extra):
                ph = ps.tile([P, 512], f32, tag="h", name="hfc")
                for ci, lt in enumerate(lhsTs):
                    nc.tensor.matmul(
                        ph[:, :dout], lhsT=lt[:],
                        rhs=wsb[Wn][:, ci * dout : (ci + 1) * dout],
                        start=(ci == 0), stop=False,
                    )
                for j, (lrow, rr_) in enumerate(extra):
                    nc.tensor.matmul(
                        ph[:, :dout], lhsT=lrow, rhs=rr_,
                        start=False, stop=(j == len(extra) - 1),
                    )
                return ph

            # ----------------------------------------------- pooling (1 pass)
            def pooling(rep):
                pw = [ps_acc.tile([P, 512], f32, tag="aggA",
                                  name=f"pw{w}_{rep}") for w in range(WWIN)]
                pe = [ps_acc.tile([P, 512], f32, tag="aggB",
                                  name=f"pe{w}_{rep}") for w in range(WWIN)]
                z1 = sb.tile([1, 512], bf16, tag="zrow")
                nc.vector.memset(z1[:], 0.0)
                for w in range(WWIN):
                    nc.tensor.matmul(pw[w][:, :512], lhsT=z1[:, :P],
                                     rhs=z1[:, :512], start=True, stop=False)
                    nc.tensor.matmul(pe[w][:, :1], lhsT=z1[:, :P],
                                     rhs=z1[:, :1], start=True, stop=False)
                for g4 in range(_ceil(NB, 4)):
                    u5g = sb.tile([P, 4, 512], bf16, tag="u5ld")
                    nb4 = min(4, NB - g4 * 4)
                    nc.sync.dma_start(
                        out=u5g[:, :nb4, :],
                        in_=u5[g4 * 4 * P : (g4 * 4 + nb4) * P, :].rearrange(
                            "(a p) w -> p a w", p=P
                        ),
                    )
                    g4pairs = [
                        jw for a_ in range(nb4)
                        for jw in pairs_by_block.get(g4 * 4 + a_, [])
                    ]
                    if not g4pairs:
                        continue
                    jmin = min(j for (j, _w) in g4pairs)
                    jnum = max(j for (j, _w) in g4pairs) - jmin + 1
                    assert jnum <= 8, f"G window too wide: {jnum}"
                    Gg = sb.tile([P, 8 * P], bf16, tag="Gg")
                    nc.sync.dma_start(
                        out=Gg[:, : jnum * P],
                        in_=G_ext[:, jmin * P : (jmin + jnum) * P],
                    )
                    for a in range(nb4):
                        b = g4 * 4 + a
                        plist = pairs_by_block.get(b, [])
                        if not plist:
                            continue
                        ub = u5g[:, a, :]
                        gm = sb.tile([P, 512], f32, tag="gatem")
                        nc.vector.tensor_tensor(
                            out=gm[:], in0=ub, in1=wsb["wgrep"][:, :],
                            op=mybir.AluOpType.mult,
                        )
                        gate = sb.tile([P, 1], f32, tag="gate")
                        nc.vector.reduce_sum(
                            out=gate[:], in_=gm[:], axis=mybir.AxisListType.X
                        )
                        e = sb.tile([P, 1], f32, tag="ecol")
                        nc.scalar.activation(
                            out=e[:], in_=gate[:],
                            func=mybir.ActivationFunctionType.Exp,
                            bias=bgrep_sb[:, :], scale=1.0,
                        )
                        e_bf = sb.tile([P, 1], bf16, tag="ebf")
                        nc.vector.tensor_copy(out=e_bf[:], in_=e[:])
                        rhs512 = sb.tile([P, 512], bf16, tag="rhs512")
                        nc.vector.tensor_scalar_mul(
                            out=rhs512[:], in0=ub, scalar1=e[:, 0:1]
                        )
                        for (j, w) in plist:
                            nc.tensor.matmul(
                                pw[w][:, :512],
                                lhsT=Gg[:, (j - jmin) * P : (j - jmin + 1) * P],
                                rhs=rhs512[:], start=False, stop=False,
                            )
                            nc.tensor.matmul(
                                pe[w][:, :1],
                                lhsT=Gg[:, (j - jmin) * P : (j - jmin + 1) * P],
                                rhs=e_bf[:], start=False, stop=False,
                            )
                for w in range(WWIN):
                    nc.tensor.matmul(pw[w][:, :512], lhsT=z1[:, :P],
                                     rhs=z1[:, :512], start=False, stop=True)
                    nc.tensor.matmul(pe[w][:, :1], lhsT=z1[:, :P],
                                     rhs=z1[:, :1], start=False, stop=True)

                    pooled = sb.tile([P, 512], f32, tag="pooled")
                    nc.vector.tensor_copy(out=pooled[:], in_=pw[w][:, :512])
                    se = sb.tile([P, 1], f32, tag="se")
                    nc.vector.tensor_tensor(
                        out=se[:], in0=pe[w][:, :1], in1=eps_col[:],
                        op=mybir.AluOpType.max,
                    )
                    si = sb.tile([P, 1], f32, tag="si")
                    nc.vector.reciprocal(out=si[:], in_=se[:])
                    fcin = sb.tile([P, 512], bf16, tag="fcin")
                    nc.vector.tensor_scalar_mul(
                        out=fcin[:], in0=pooled[:], scalar1=si[:, 0:1]
                    )
                    l1 = main_matmul(
                        transpose_chunks(fcin, 512), "Wf1", 256,
                        [(ones_row[:, :], wsb["cf1"][:, :256])],
                    )
                    h1 = lrelu(l1, 256, "fc1")
                    l2 = main_matmul(
                        transpose_chunks(h1, 256), "Wf2", 128,
                        [(ones_row[:, :], wsb["cf2"][:, :128])],
                    )
                    h2 = lrelu(l2, 128, "fc2")
                    l3 = main_matmul(
                        transpose_chunks(h2, 128), "Wf3", 1,
                        [(ones_row[:, :], wsb["cf3"][:, :1])],
                    )
                    oc = sb.tile([P, 1], f32, tag="oc")
                    nc.vector.tensor_copy(out=oc[:], in_=l3[:, :1])
                    nc.sync.dma_start(
                        out=out_ext[w * P : (w + 1) * P, :], in_=oc[:]
                    )

            # ----------------- the program -----------------
            for rep in range(REPS):
                pool0build()
                conv1()
                if PHASES >= 2:
                    conv_postA(u1T, 4, "W2", 256, "c2", h2s, "L2")
                    sendbuild(h2s, sendA, 256)
                    a2a(sendA, pool2)
                    conv_postB_T(pool2, h2s, 256, "b2", u2T, "L2")
                if PHASES >= 3:
                    conv_postA(u2T, 2, "W3", 128, "c3", h3s, "L3")
                    sendbuild(h3s, sendB, 128)
                    a2a(sendB, pool3)
                    conv_postB_N(pool3, h3s, 128, "b3", u3, "L3")
                if PHASES >= 4:
                    sendbuild(u3, sendB, 128)
                    a2a(sendB, poolu3)
                    conv_pre(poolu3, u3, 128, "W4", 256, "c4r", "c4",
                             u4, "L4")
                    sendbuild(u4, sendA, 256)
                    a2a(sendA, poolu4)
                    conv_pre(poolu4, u4, 256, "W5", 512, "c5r", "c5",
                             u5, "L5")
                if PHASES >= 5:
                    pooling(rep)

    nc.finalize()
    return nc


# ----------------------------------------------------------------- frontend

_CACHE = {}


def _prepare(inputs, B):
    x = np.asarray(inputs["x"], np.float32)
    ei = np.asarray(inputs["edge_index"], np.int64)
    ea = np.asarray(inputs["edge_attr"], np.float32)
    bt = np.asarray(inputs["batch"], np.int64)
    key = hash((x.shape, ei.tobytes(), bt.tobytes(), B))
    if key not in _CACHE:
        meta, per_core, x0p = _preprocess(x, ei, ea, bt, B)
        nc = build_program(meta)
        _CACHE.clear()
        _CACHE[key] = (meta, per_core, x0p, nc)
    return _CACHE[key]


def _in_maps(meta, per_core, x0p, wf):
    maps = []
    for c in range(NCORES):
        m = dict(x0=x0p, bgrep=wf["bgrep"], **{
            n: wf[n] for n, _ in WSHAPES
        })
        m["S2"] = per_core[c]["S2"]
        m["gidx2"] = per_core[c]["gidx2"]
        m["sidx"] = per_core[c]["sidx"]
        m["sidx0"] = per_core[c]["sidx0"]
        m["x0loc"] = per_core[c]["x0loc"]
        m["rrow"] = per_core[c]["rrow"]
        m["G"] = per_core[c]["G"]
        m["D"] = per_core[c]["D"]
        maps.append(m)
    return maps


def _assemble(meta, results, inputs, B):
    GPC, WWIN = meta["GPC"], meta["WWIN"]
    out = np.empty(B, np.float32)
    for c in range(NCORES):
        out[c * GPC : (c + 1) * GPC] = results[c]["out"][:GPC, 0]
    cnt = np.bincount(np.asarray(inputs["batch"], np.int64), minlength=B)
    if (cnt == 0).any():
        Wf1, bf1 = np.asarray(inputs["Wf1"]), np.asarray(inputs["bf1"])
        Wf2, bf2 = np.asarray(inputs["Wf2"]), np.asarray(inputs["bf2"])
        Wf3, bf3 = np.asarray(inputs["Wf3"]), np.asarray(inputs["bf3"])
        lr = lambda z: np.where(z >= 0, z, 0.01 * z)
        h = lr(np.zeros(Wf1.shape[0]) @ Wf1 + bf1)
        h = lr(h @ Wf2 + bf2)
        out[cnt == 0] = float(h @ Wf3 + bf3)
    return out


def kernel(_B=B_DEFAULT, **inputs):
    meta, per_core, x0p, nc = _prepare(inputs, _B)
    wf = _fold_weights(inputs)
    maps = _in_maps(meta, per_core, x0p, wf)
    res = run_bass_kernel_spmd(nc, maps, core_ids=list(range(NCORES)))
    return _assemble(meta, res.results, inputs, _B)


# revision 25
# speedup vs baseline: 2.3442x; 1.4522x over previous
"""GCNNet forward on 8 Trainium2 NeuronCores (Bass/Tile SPMD), v2.

Strategy
--------
- Nodes partitioned graph-aligned across 8 cores (B/8 graphs per core).
- Edge aggregation (GCN symmetric norm) via one-hot matmuls in SWAPPED
  orientation: accT[feat, dst] += gathered[slot, feat]^T-free @ S[slot, dst].
  The PSUM result is directly the lhsT for the next dense matmul, so the
  conv stack needs NO PE transposes.
- Self-loops are NOT gathered: per dst block, lhsT = contiguous local
  feature block, rhs = diag(dinv^2) tile (D). Removes ~30% of gather rows.
- L1 gathers x at its true width 64.
- u1/u2 are stored transposed ([feat, node]) so the following matmul-first
  layers (L2/L3 postA) read lhsT chunks directly. h2/h3/u3/u4 stay
  node-major for AllGather + gather.
- BatchNorm affine + biases folded into weights host-side (rank-1 rows
  with rrow[n] = row-sum of norm incl self).
- Cross-core exchange: 4 bf16 AllGathers (h2, h3, u3, u4).
- Attention pooling: single pass over u5; per-graph one-hot matmuls into
  per-window PSUM accumulators, only (block, window) pairs that are
  nonzero on some core. Softmax without max-subtraction; host fixes
  empty graphs.
"""
import os
import sys

for _p in ("/opt/trn_rl_repo", "/root/.axon_site/_ro/trn_rl_repo"):
    if os.path.isdir(_p) and _p not in sys.path:
        sys.path.insert(0, _p)

import numpy as np
import ml_dtypes

import concourse.bass as bass
import concourse.bacc as bacc
import concourse.mybir as mybir
import concourse.tile as tile
from concourse.bass_utils import run_bass_kernel_spmd
from concourse.masks import make_identity

P = 128
NCORES = 8
NRANGE = 4
GBLK = 1  # blocks per gather group (HW dma_gather breaks with large chunks)
AGRP = 2  # dst blocks per agg dma_gather (>2 crashes HW: chunk too large)

bf16 = mybir.dt.float16  # working dtype (fp16: 10-bit mantissa, safe ranges)
f32 = mybir.dt.float32
i16 = mybir.dt.int16
BF = np.float16

B_DEFAULT = 2048
PHASES = 5  # debug: how many phases of the program to emit
REPS = 1  # timing: repeat the whole body REPS times inside the program
MAXG = 10**9


def set_f32_debug():
    global bf16, BF
    bf16 = mybir.dt.float32
    BF = np.float32


def set_f16():
    global bf16, BF
    bf16 = mybir.dt.float16
    BF = np.float16


def _ceil(a, b):
    return -(-a // b)


# ----------------------------------------------------------------- host prep

def _preprocess(x, edge_index, edge_attr, batch, B):
    N = x.shape[0]
    GPC = B // NCORES
    src = np.asarray(edge_index[0], np.int64)
    dst = np.asarray(edge_index[1], np.int64)
    ew = np.asarray(edge_attr, np.float64)
    batch = np.asarray(batch, np.int64)

    gstarts = np.searchsorted(batch, np.arange(0, B + 1, GPC))
    node_start = gstarts[:-1]
    node_cnt = np.diff(gstarts)
    Np = int(_ceil(max(int(node_cnt.max()), 1), P) * P)
    assert 2 * Np <= 32767, f"Np={Np} too large for int16 gather ranges"
    NB = Np // P
    RSZ = 2 * Np

    core_of = batch // GPC
    pid = core_of * Np + (np.arange(N) - node_start[core_of])
    local_graph = batch - core_of * GPC

    deg = np.bincount(dst, weights=ew, minlength=N) + 1.0
    dinv = 1.0 / np.sqrt(deg)
    dinv2 = dinv * dinv
    norm_e = dinv[src] * ew * dinv[dst]
    rvec = np.bincount(dst, weights=norm_e, minlength=N) + dinv2

    # edges only (self-loops handled via D diag tiles)
    es, ed, en = src, dst, norm_e

    e_core = core_of[ed]
    e_block = (pid[ed] % Np) // P
    e_dl = pid[ed] % P

    WWIN = _ceil(GPC, P)

    # ---- selective exchange (AllToAll): per (sender, receiver) row lists
    s_core_e = core_of[es]
    l_src = pid[es] % Np  # sender-local row id
    sendlists = [[None] * NCORES for _ in range(NCORES)]
    maxlen = 1
    for s in range(NCORES):
        for r in range(NCORES):
            mm = (s_core_e == s) & (e_core == r)
            u = np.unique(l_src[mm])
            sendlists[s][r] = u
            maxlen = max(maxlen, len(u))
    CH = _ceil(maxlen, 64) * 64  # 2*CH must be a multiple of 128
    NSEND = NCORES * CH
    NSEND = _ceil(NSEND, P) * P
    assert NSEND <= 32767, f"NSEND={NSEND} exceeds int16 gather range"
    NST = NSEND // P  # send tiles

    # pool row for each edge: s*CH + position in sendlists[s][r]
    pool_row = np.empty(len(es), np.int64)
    for s in range(NCORES):
        for r in range(NCORES):
            mm = (s_core_e == s) & (e_core == r)
            if mm.any():
                pool_row[mm] = s * CH + np.searchsorted(
                    sendlists[s][r], l_src[mm]
                )

    # single-range bucketing for the 4 exchanged layers
    key2 = (e_core * NB + e_block).astype(np.int64)
    cnt2 = np.bincount(key2, minlength=NCORES * NB).reshape(NCORES, NB)
    K2 = _ceil(cnt2.max(axis=0), P)  # [NB]
    tile_of_b2 = np.concatenate(([0], np.cumsum(K2)))[:-1]
    ntilesE = int(K2.sum())
    order2 = np.lexsort((e_block, e_core))
    excl2 = np.concatenate(
        ([0], np.cumsum(np.bincount(key2, minlength=NCORES * NB)))
    )
    pos2 = np.arange(len(order2)) - excl2[key2[order2]]
    slot2 = tile_of_b2[e_block[order2]] * P + pos2

    # pooling (block, window) pairs: union over cores of nonzero G tiles
    lg_all = []
    pair_set = set()
    for c in range(NCORES):
        lg = np.full(Np, -1, np.int64)
        ncnt = int(node_cnt[c])
        lg[:ncnt] = local_graph[node_start[c] : node_start[c] + ncnt]
        lg_all.append(lg)
        for b in range(NB):
            seg = lg[b * P : (b + 1) * P]
            for w in np.unique(seg[seg >= 0] // P):
                pair_set.add((b, int(w)))
    pairs = sorted(pair_set)
    npairs = len(pairs)
    pairs_by_block = {}
    for j, (b, w) in enumerate(pairs):
        pairs_by_block.setdefault(b, []).append((j, w))

    G_all, D_all = [], []
    S2_all, idx2_all, sidx_all, sidx0_all = [], [], [], []
    rrow = np.zeros((NCORES, Np), np.float32)
    for c in range(NCORES):
        # exchanged-layer S2/gidx2 (single range into the A2A pool)
        S2 = np.zeros((ntilesE, P, P), np.float32)
        idx2_lin = np.zeros(ntilesE * P, np.int16)
        m2 = e_core[order2] == c
        sl2 = slot2[m2]
        S2[sl2 // P, sl2 % P, e_dl[order2][m2]] = en[order2][m2]
        idx2_lin[sl2] = pool_row[order2][m2].astype(np.int16)
        S2_all.append(
            np.ascontiguousarray(S2.transpose(1, 0, 2))
            .reshape(P, ntilesE * P).astype(BF)
        )
        p2 = idx2_lin.reshape(-1, 16).T
        idx2_all.append(np.tile(p2, (8, 1)))

        sidx_lin = np.zeros(NSEND, np.int16)
        for r in range(NCORES):
            L = sendlists[c][r]
            sidx_lin[r * CH : r * CH + len(L)] = L.astype(np.int16)
        ps_ = sidx_lin.reshape(-1, 16).T
        sidx_all.append(np.tile(ps_, (8, 1)))

        # pool0 build indices: row s*CH+i <- x0[s*Np + sendlists[s][c][i]],
        # gathered per range r'=s//2 from x0[r'*RSZ:(r'+1)*RSZ]
        sidx0_lin = np.zeros(NSEND, np.int16)
        for s in range(NCORES):
            L = sendlists[s][c]
            sidx0_lin[s * CH : s * CH + len(L)] = (
                (s % 2) * Np + L
            ).astype(np.int16)
        p0_ = sidx0_lin.reshape(-1, 16).T
        sidx0_all.append(np.tile(p0_, (8, 1)))

        ncnt = int(node_cnt[c])
        rrow[c, :ncnt] = rvec[node_start[c] : node_start[c] + ncnt]

        D = np.zeros((NB, P, P), np.float32)
        dloc = dinv2[node_start[c] : node_start[c] + ncnt]
        nn = np.arange(ncnt)
        D[nn // P, nn % P, nn % P] = dloc
        D_all.append(
            np.ascontiguousarray(D.transpose(1, 0, 2))
            .reshape(P, NB * P).astype(BF)
        )

        G = np.zeros((npairs, P, P), np.float32)
        lg = lg_all[c]
        for j, (b, w) in enumerate(pairs):
            seg = lg[b * P : (b + 1) * P]
            v = (seg >= 0) & (seg // P == w)
            G[j, np.nonzero(v)[0], seg[v] - w * P] = 1.0
        G_all.append(
            np.ascontiguousarray(G.transpose(1, 0, 2))
            .reshape(P, npairs * P).astype(BF)
        )

    IN = x.shape[1]
    # dma_gather requires elem_size to be a multiple of 256B -> pad x to 128
    x0p = np.zeros((NCORES * Np, P), np.float32)
    x0p[pid, :IN] = np.asarray(x, np.float32)
    x0p = x0p.astype(BF)
    x0loc = [x0p[c * Np : (c + 1) * Np] for c in range(NCORES)]

    meta = dict(
        N=N, B=B, GPC=GPC, Np=Np, NB=NB, RSZ=RSZ, WWIN=WWIN,
        node_start=node_start, node_cnt=node_cnt,
        IN=IN, npairs=npairs, pairs=pairs, pairs_by_block=pairs_by_block,
        CH=CH, NSEND=NSEND, NST=NST, K2=K2, tile_of_b2=tile_of_b2,
        ntilesE=ntilesE,
    )
    per_core = [
        dict(x0loc=x0loc[c],
             rrow=rrow[c].astype(BF)[None, :], G=G_all[c], D=D_all[c],
             S2=S2_all[c], gidx2=idx2_all[c], sidx=sidx_all[c],
             sidx0=sidx0_all[c])
        for c in range(NCORES)
    ]
    return meta, per_core, x0p


def _fold_weights(inp):
    f = lambda k: np.asarray(inp[k], np.float64)
    A, Bb = [], []
    for i in range(1, 6):
        a = f("g%d" % i) / np.sqrt(f("v%d" % i) + 1e-5)
        A.append(a)
        Bb.append(f("be%d" % i) - f("m%d" % i) * a)

    def pack(W):
        din, dout = W.shape
        nch = _ceil(din, P)
        Wp = np.zeros((nch * P, dout))
        Wp[:din] = W
        return (
            np.ascontiguousarray(Wp.reshape(nch, P, dout).transpose(1, 0, 2))
            .reshape(P, nch * dout).astype(BF)
        )

    o = {}
    o["W1"] = pack(f("W1"))
    o["c1"] = f("b1")[None, :].astype(BF)
    o["W2"] = pack(A[0][:, None] * f("W2"))
    o["c2"] = (Bb[0] @ f("W2"))[None, :].astype(BF)
    o["b2"] = f("b2")[None, :].astype(BF)
    o["W3"] = pack(A[1][:, None] * f("W3"))
    o["c3"] = (Bb[1] @ f("W3"))[None, :].astype(BF)
    o["b3"] = f("b3")[None, :].astype(BF)
    o["W4"] = pack(A[2][:, None] * f("W4"))
    o["c4r"] = (Bb[2] @ f("W4"))[None, :].astype(BF)
    o["c4"] = f("b4")[None, :].astype(BF)
    o["W5"] = pack(A[3][:, None] * f("W5"))
    o["c5r"] = (Bb[3] @ f("W5"))[None, :].astype(BF)
    o["c5"] = f("b5")[None, :].astype(BF)
    wg = A[4] * f("Wg")[:, 0]
    o["wgrep"] = np.tile(wg[None, :], (P, 1)).astype(BF)
    o["bgrep"] = np.full(
        (P, 1), float(Bb[4] @ f("Wg")[:, 0] + f("bg")[0]), np.float32
    )
    o["Wf1"] = pack(A[4][:, None] * f("Wf1"))
    o["cf1"] = (f("bf1") + Bb[4] @ f("Wf1"))[None, :].astype(BF)
    o["Wf2"] = pack(f("Wf2"))
    o["cf2"] = f("bf2")[None, :].astype(BF)
    o["Wf3"] = pack(f("Wf3"))
    o["cf3"] = f("bf3")[None, :].astype(BF)
    return o


WSHAPES = [
    ("W1", [P, 512]), ("c1", [1, 512]),
    ("W2", [P, 4 * 256]), ("c2", [1, 256]), ("b2", [1, 256]),
    ("W3", [P, 2 * 128]), ("c3", [1, 128]), ("b3", [1, 128]),
    ("W4", [P, 256]), ("c4r", [1, 256]), ("c4", [1, 256]),
    ("W5", [P, 2 * 512]), ("c5r", [1, 512]), ("c5", [1, 512]),
    ("wgrep", [P, 512]), ("Wf1", [P, 4 * 256]), ("cf1", [1, 256]),
    ("Wf2", [P, 2 * 128]), ("cf2", [1, 128]),
    ("Wf3", [P, 1]), ("cf3", [1, 1]),
]


# ------------------------------------------------------------- device build

def build_program(meta):
    Np, NB, RSZ = meta["Np"], meta["NB"], meta["RSZ"]
    WWIN, IN = meta["WWIN"], meta["IN"]
    npairs, pairs_by_block = meta["npairs"], meta["pairs_by_block"]
    NSEND, NST = meta["NSEND"], meta["NST"]
    K2, tile_of_b2, ntilesE = meta["K2"], meta["tile_of_b2"], meta["ntilesE"]
    NPT = NCORES * Np

    nc = bacc.Bacc(None)
    dp = nc.declare_dram_parameter
    x0_ext = dp("x0", [NPT, P], bf16, isOutput=False)
    x0loc_ext = dp("x0loc", [Np, P], bf16, isOutput=False)
    S2_ext = dp("S2", [P, ntilesE * P], bf16, isOutput=False)
    gidx2_ext = dp("gidx2", [P, ntilesE * 8], i16, isOutput=False)
    sidx_ext = dp("sidx", [P, NST * 8], i16, isOutput=False)
    sidx0_ext = dp("sidx0", [P, NST * 8], i16, isOutput=False)
    rrow_ext = dp("rrow", [1, Np], bf16, isOutput=False)
    G_ext = dp("G", [P, npairs * P], bf16, isOutput=False)
    D_ext = dp("D", [P, NB * P], bf16, isOutput=False)
    wext = {n: dp(n, sh, bf16, isOutput=False) for n, sh in WSHAPES}
    bgrep_ext = dp("bgrep", [P, 1], f32, isOutput=False)
    out_ext = dp("out", [WWIN * P, 1], f32, isOutput=True)

    u1T = nc.dram_tensor("u1T", [4 * P, Np], bf16)
    u2T = nc.dram_tensor("u2T", [2 * P, Np], bf16)
    h2s = nc.dram_tensor("h2s", [Np, 256], bf16)
    h3s = nc.dram_tensor("h3s", [Np, 128], bf16)
    u3 = nc.dram_tensor("u3", [Np, 128], bf16)
    u4 = nc.dram_tensor("u4", [Np, 256], bf16)
    u5 = nc.dram_tensor("u5", [Np, 512], bf16)
    pool0 = nc.dram_tensor("pool0", [NSEND, P], bf16)
    sendA = nc.dram_tensor("sendA", [NSEND, 256], bf16)
    sendB = nc.dram_tensor("sendB", [NSEND, 128], bf16)
    pool2 = nc.dram_tensor("pool2", [NSEND, 256], bf16)
    pool3 = nc.dram_tensor("pool3", [NSEND, 128], bf16)
    poolu3 = nc.dram_tensor("poolu3", [NSEND, 128], bf16)
    poolu4 = nc.dram_tensor("poolu4", [NSEND, 256], bf16)
    RG = [list(range(NCORES))]

    with tile.TileContext(nc) as tc:
        with (
            tc.tile_pool(name="persist", bufs=1) as pp,
            tc.tile_pool(name="sb", bufs=2) as sb,
            tc.tile_pool(name="sb3", bufs=3) as sb3,
            tc.tile_pool(name="ps", bufs=2, space="PSUM") as ps,
            tc.tile_pool(name="ps_acc", bufs=2, space="PSUM") as ps_acc,
        ):
            gidx2_sb = pp.tile([P, ntilesE * 8], i16)
            nc.sync.dma_start(out=gidx2_sb[:], in_=gidx2_ext[:, :])
            sidx_sb = pp.tile([P, NST * 8], i16)
            nc.sync.dma_start(out=sidx_sb[:], in_=sidx_ext[:, :])
            sidx0_sb = pp.tile([P, NST * 8], i16)
            nc.sync.dma_start(out=sidx0_sb[:], in_=sidx0_ext[:, :])
            wsb = {}
            for n, sh in WSHAPES:
                wsb[n] = pp.tile(sh, bf16, tag="w_" + n, name="w_" + n)
                nc.sync.dma_start(out=wsb[n][:], in_=wext[n][:, :])
            bgrep_sb = pp.tile([P, 1], f32)
            nc.sync.dma_start(out=bgrep_sb[:], in_=bgrep_ext[:, :])
            D_sb = pp.tile([P, NB * P], bf16, name="D_sb")
            nc.sync.dma_start(out=D_sb[:], in_=D_ext[:, :])
            rrow_sb = pp.tile([1, Np], bf16, name="rrow_sb")
            nc.sync.dma_start(out=rrow_sb[:], in_=rrow_ext[:, :])
            ident = pp.tile([P, P], bf16)
            make_identity(nc, ident[:])
            ones_row = pp.tile([1, P], bf16)
            nc.vector.memset(ones_row[:], 1.0)
            eps_col = pp.tile([P, 1], f32)
            nc.vector.memset(eps_col[:], 1e-20)

            def pool0build():
                """local build of the L1 pool from the replicated x0:
                sender s's chunk lies in range s//2 of x0."""
                GW = 8
                ntr = NST // 4  # tiles per range (2*CH rows)
                for rp in range(4):
                    base = rp * ntr
                    for gg in range(_ceil(ntr, GW)):
                        nt = min(GW, ntr - gg * GW)
                        t = sb.tile([P, GW, P], bf16, tag="p0b",
                                    name=f"p0b_{rp}_{gg}")
                        nc.gpsimd.dma_gather(
                            out_ap=t[:, :nt, :],
                            in_ap=x0_ext[rp * RSZ : (rp + 1) * RSZ, :],
                            idxs_ap=sidx0_sb[
                                :, (base + gg * GW) * 8 : (base + gg * GW + nt) * 8
                            ],
                            num_idxs=nt * P,
                            num_idxs_reg=nt * P,
                            elem_size=P,
                        )
                        nc.sync.dma_start(
                            out=pool0[
                                (base + gg * GW) * P : (base + gg * GW + nt) * P, :
                            ].rearrange("(a p) w -> p a w", p=P),
                            in_=t[:, :nt, :],
                        )

            NTG4 = max(
                sum(int(K2[b]) for b in range(g0, min(g0 + AGRP, NB)))
                for g0 in range(0, NB, AGRP)
            )

            def gather_pool_grp(b0, pool, w):
                """one gather + S load covering blocks b0..b0+AGRP-1."""
                b1 = min(b0 + AGRP, NB)
                t0 = int(tile_of_b2[b0])
                nt = sum(int(K2[b]) for b in range(b0, b1))
                gall = sb.tile([P, NTG4, w], bf16, tag=f"gpw{w}",
                               name=f"gp_{b0}")
                nc.gpsimd.dma_gather(
                    out_ap=gall[:, :nt, :],
                    in_ap=pool[0:NSEND, :w],
                    idxs_ap=gidx2_sb[:, t0 * 8 : (t0 + nt) * 8],
                    num_idxs=nt * P,
                    num_idxs_reg=nt * P,
                    elem_size=w,
                )
                s_grp = sb.tile([P, NTG4 * P], bf16, tag="sgrp2",
                                name=f"sg2_{b0}")
                nc.sync.dma_start(
                    out=s_grp[:, : nt * P],
                    in_=S2_ext[:, t0 * P : (t0 + nt) * P],
                )
                return (gall, s_grp, t0)

            def gather_for(cache, b, pool, w):
                b0 = (b // AGRP) * AGRP
                if cache.get("b0") != b0:
                    cache["b0"] = b0
                    cache["gts"] = gather_pool_grp(b0, pool, w)
                return block_tiles(cache["gts"], b)

            def block_tiles(gts, b):
                gall, s_grp, g_t0 = gts
                t0 = int(tile_of_b2[b])
                return (gall, s_grp, g_t0,
                        [t0 + k for k in range(int(K2[b]))])

            def sendbuild(src_loc, send_dram, w):
                """gather own rows every peer needs into the A2A send buf."""
                GW = 8
                for gg in range(_ceil(NST, GW)):
                    nt = min(GW, NST - gg * GW)
                    t = sb.tile([P, GW, w], bf16, tag=f"snd{w}",
                                name=f"snd{w}_{gg}")
                    nc.gpsimd.dma_gather(
                        out_ap=t[:, :nt, :],
                        in_ap=src_loc[0:Np, :w],
                        idxs_ap=sidx_sb[:, gg * GW * 8 : (gg * GW + nt) * 8],
                        num_idxs=nt * P,
                        num_idxs_reg=nt * P,
                        elem_size=w,
                    )
                    nc.sync.dma_start(
                        out=send_dram[gg * GW * P : (gg * GW + nt) * P, :]
                        .rearrange("(a p) w -> p a w", p=P),
                        in_=t[:, :nt, :],
                    )

            def a2a(src, dst):
                nc.gpsimd.collective_compute(
                    "AllToAll", mybir.AluOpType.bypass,
                    replica_groups=RG, ins=[src[:]], outs=[dst[:]],
                )

            def load_self4(src_loc, g4, w, tag):
                """contiguous rows for blocks 4*g4 .. 4*g4+3 -> [P, 4, w]."""
                nb4 = min(4, NB - g4 * 4)
                t = sb.tile([P, 4, w], bf16, tag=tag)
                nc.sync.dma_start(
                    out=t[:, :nb4, :],
                    in_=src_loc[g4 * 4 * P : (g4 * 4 + nb4) * P, :w].rearrange(
                        "(a p) w -> p a w", p=P
                    ),
                )
                return t

            def seg_aggT(b, gts, w, self_sb, bias_row):
                """swapped-orientation aggregation: accT[feat, dst].

                Returns list of PSUM tiles, one per 128-chunk of w. w may be
                smaller than the gathered width (trailing cols ignored).
                self_sb: [P, >=w] node-major contiguous self rows for block b.
                bias_row: [1, w] row to add per-dst (or None).
                """
                gall, s_grp, g_t0, tiles = gts
                nchunk = _ceil(w, P)
                accs = []
                for c in range(nchunk):
                    cw = min(P, w - c * P)
                    acc = ps_acc.tile([P, 512], f32, tag=f"agg{'AB'[c]}",
                                      name=f"agg{c}")
                    first = True
                    for tg in tiles:
                        nc.tensor.matmul(
                            acc[:cw, :P],
                            lhsT=gall[:, tg - g_t0, c * P : c * P + cw],
                            rhs=s_grp[:, (tg - g_t0) * P : (tg - g_t0 + 1) * P],
                            start=first, stop=False,
                        )
                        first = False
                    last = bias_row is None
                    nc.tensor.matmul(
                        acc[:cw, :P],
                        lhsT=self_sb[:, c * P : c * P + cw],
                        rhs=D_sb[:, b * P : (b + 1) * P],
                        start=first, stop=last,
                    )
                    if bias_row is not None:
                        nc.tensor.matmul(
                            acc[:cw, :P],
                            lhsT=bias_row[:, c * P : c * P + cw],
                            rhs=ones_row[:, :],
                            start=False, stop=True,
                        )
                    accs.append((acc, cw))
                return accs

            def accs_to_sbuf(accs, tag):
                outs = []
                for i, (acc, cw) in enumerate(accs):
                    st = sb.tile([P, P], bf16, tag=f"{tag}{i}")
                    nc.vector.tensor_copy(out=st[:cw, :], in_=acc[:cw, :P])
                    outs.append((st, cw))
                return outs

            def lrelu(psum, w, tag):
                u = sb.tile([P, w], bf16, tag=f"u{tag}")
                nc.scalar.activation(
                    out=u[:], in_=psum[:, :w],
                    func=mybir.ActivationFunctionType.Lrelu, alpha=0.01,
                )
                return u

            # -------------------------------------------------- L1 conv_pre
            def conv1():
                gc1 = {}
                for g4 in range(_ceil(NB, 4)):
                    selfs = load_self4(x0loc_ext, g4, P, "self1")
                    stages = [
                        sb.tile([P, 4 * P], bf16, tag=f"u1st{oc}",
                                name=f"u1st{oc}")
                        for oc in range(4)
                    ]
                    for a in range(4):
                        b = g4 * 4 + a
                        if b >= NB:
                            break
                        gts = gather_for(gc1, b, pool0, P)
                        accs = seg_aggT(b, gts, IN, selfs[:, a, :], None)
                        aggT = accs_to_sbuf(accs, "aggT1")[0]
                        st0, cw = aggT
                        for oc in range(4):
                            phT = ps.tile([P, 512], f32, tag="h",
                                          name="phT")
                            nc.tensor.matmul(
                                phT[:, :P],
                                lhsT=wsb["W1"][:cw, oc * P : (oc + 1) * P],
                                rhs=st0[:cw, :],
                                start=True, stop=False,
                            )
                            nc.tensor.matmul(
                                phT[:, :P],
                                lhsT=wsb["c1"][:, oc * P : (oc + 1) * P],
                                rhs=ones_row[:, :],
                                start=False, stop=True,
                            )
                            nc.scalar.activation(
                                out=stages[oc][:, a * P : (a + 1) * P],
                                in_=phT[:, :P],
                                func=mybir.ActivationFunctionType.Lrelu,
                                alpha=0.01,
                            )
                    nb4 = min(4, NB - g4 * 4)
                    for oc in range(4):
                        nc.sync.dma_start(
                            out=u1T[oc * P : (oc + 1) * P,
                                    g4 * 4 * P : (g4 * 4 + nb4) * P],
                            in_=stages[oc][:, : nb4 * P],
                        )

            # ---------------------------------------- postA: h = u @ W + c
            def conv_postA(uT_src, nch, Wn, dout, cn, hdst, tag):
                for g4 in range(_ceil(NB, 4)):
                    nb4g = min(4, NB - g4 * 4)
                    ins = []
                    for ic in range(nch):
                        t = sb.tile([P, 4 * P], bf16, tag=f"pA{tag}{ic}")
                        nc.sync.dma_start(
                            out=t[:, : nb4g * P],
                            in_=uT_src[ic * P : (ic + 1) * P,
                                       g4 * 4 * P : (g4 * 4 + nb4g) * P],
                        )
                        ins.append(t)
                    for a in range(4):
                        b = g4 * 4 + a
                        if b >= NB:
                            break
                        ph = ps.tile([P, 512], f32, tag="h", name=f"h{tag}")
                        for ic in range(nch):
                            nc.tensor.matmul(
                                ph[:, :dout],
                                lhsT=ins[ic][:, a * P : (a + 1) * P],
                                rhs=wsb[Wn][:, ic * dout : (ic + 1) * dout],
                                start=(ic == 0), stop=False,
                            )
                        nc.tensor.matmul(
                            ph[:, :dout], lhsT=ones_row[:, :],
                            rhs=wsb[cn][:, :dout],
                            start=False, stop=True,
                        )
                        hb = sb.tile([P, dout], bf16, tag=f"hst{tag}")
                        nc.vector.tensor_copy(out=hb[:], in_=ph[:, :dout])
                        nc.sync.dma_start(
                            out=hdst[b * P : (b + 1) * P, :], in_=hb[:]
                        )

            # ------------------- postB: agg(h) + b -> lrelu -> uT (swapped)
            def conv_postB_T(pool, src_loc, w, bn, uT_dst, tag):
                nchunk = w // P
                gcB = {}
                for g4 in range(_ceil(NB, 4)):
                    selfs = load_self4(src_loc, g4, w, f"selfB{tag}")
                    stages = [
                        sb.tile([P, 4 * P], bf16, tag=f"uB{tag}{c}",
                                name=f"uB{tag}{c}")
                        for c in range(nchunk)
                    ]
                    for a in range(4):
                        b = g4 * 4 + a
                        if b >= NB:
                            break
                        gts = gather_for(gcB, b, pool, w)
                        accs = seg_aggT(b, gts, w, selfs[:, a, :],
                                        wsb[bn][:, :w])
                        for c, (acc, cw) in enumerate(accs):
                            nc.scalar.activation(
                                out=stages[c][:, a * P : (a + 1) * P],
                                in_=acc[:cw, :P],
                                func=mybir.ActivationFunctionType.Lrelu,
                                alpha=0.01,
                            )
                    nb4 = min(4, NB - g4 * 4)
                    for c in range(nchunk):
                        nc.sync.dma_start(
                            out=uT_dst[c * P : (c + 1) * P,
                                       g4 * 4 * P : (g4 * 4 + nb4) * P],
                            in_=stages[c][:, : nb4 * P],
                        )

            # ------- postB normal orientation: agg(h) + b -> lrelu -> u [Np,w]
            def conv_postB_N(pool, src_loc, w, bn, u_dst, tag):
                gcN = {}
                for g4 in range(_ceil(NB, 4)):
                    selfs = load_self4(src_loc, g4, w, f"selfN{tag}")
                    stage = sb.tile([P, 4, w], bf16, tag=f"uN{tag}")
                    for a in range(4):
                        b = g4 * 4 + a
                        if b >= NB:
                            break
                        gall, s_grp, g_t0, tiles = gather_for(gcN, b, pool, w)
                        acc = ps_acc.tile([P, 512], f32, tag="aggA",
                                          name="aggN")
                        first = True
                        for tg in tiles:
                            nc.tensor.matmul(
                                acc[:, :w],
                                lhsT=s_grp[:, (tg - g_t0) * P : (tg - g_t0 + 1) * P],
                                rhs=gall[:, tg - g_t0, :],
                                start=first, stop=False,
                            )
                            first = False
                        nc.tensor.matmul(
                            acc[:, :w], lhsT=D_sb[:, b * P : (b + 1) * P],
                            rhs=selfs[:, a, :], start=first, stop=False,
                        )
                        nc.tensor.matmul(
                            acc[:, :w], lhsT=ones_row[:, :], rhs=wsb[bn][:, :w],
                            start=False, stop=True,
                        )
                        nc.scalar.activation(
                            out=stage[:, a, :], in_=acc[:, :w],
                            func=mybir.ActivationFunctionType.Lrelu,
                            alpha=0.01,
                        )
                    nb4 = min(4, NB - g4 * 4)
                    nc.sync.dma_start(
                        out=u_dst[g4 * 4 * P : (g4 * 4 + nb4) * P, :]
                        .rearrange("(a p) w -> p a w", p=P),
                        in_=stage[:, :nb4, :],
                    )

            # -------- conv_pre (L4/L5): agg(u) swapped, then main matmul
            def conv_pre(pool, src_loc, w, Wn, dout, crn, cn, u_dst, tag):
                gcP = {}
                for g4 in range(_ceil(NB, 4)):
                    selfs = load_self4(src_loc, g4, w, f"selfP{tag}")
                    stage = sb.tile([P, 4, dout], bf16, tag=f"uP{tag}")
                    for a in range(4):
                        b = g4 * 4 + a
                        if b >= NB:
                            break
                        gts = gather_for(gcP, b, pool, w)
                        accs = seg_aggT(b, gts, w, selfs[:, a, :], None)
                        aggTs = accs_to_sbuf(accs, f"agg{tag}")
                        ph = ps.tile([P, 512], f32, tag="h", name=f"hP{tag}")
                        for c, (st, cw) in enumerate(aggTs):
                            nc.tensor.matmul(
                                ph[:, :dout],
                                lhsT=st[:cw, :],
                                rhs=wsb[Wn][:cw, c * dout : (c + 1) * dout],
                                start=(c == 0), stop=False,
                            )
                        nc.tensor.matmul(
                            ph[:, :dout],
                            lhsT=rrow_sb[:, b * P : (b + 1) * P],
                            rhs=wsb[crn][:, :dout],
                            start=False, stop=False,
                        )
                        nc.tensor.matmul(
                            ph[:, :dout], lhsT=ones_row[:, :],
                            rhs=wsb[cn][:, :dout],
                            start=False, stop=True,
                        )
                        nc.scalar.activation(
                            out=stage[:, a, :], in_=ph[:, :dout],
                            func=mybir.ActivationFunctionType.Lrelu,
                            alpha=0.01,
                        )
                    nb4 = min(4, NB - g4 * 4)
                    nc.sync.dma_start(
                        out=u_dst[g4 * 4 * P : (g4 * 4 + nb4) * P, :]
                        .rearrange("(a p) w -> p a w", p=P),
                        in_=stage[:, :nb4, :],
                    )

            def transpose_chunks(src_sb, w):
                outs = []
                for ci in range(w // P):
                    pt = ps.tile([P, P], bf16, tag="trps")
                    nc.tensor.transpose(
                        out=pt[:], in_=src_sb[:, ci * P : (ci + 1) * P],
                        identity=ident[:],
                    )
                    st = sb.tile([P, P], bf16, tag="trsb")
                    nc.vector.tensor_copy(out=st[:], in_=pt[:])
                    outs.append(st)
                return outs

            def main_matmul(lhsTs, Wn, dout, extra):
                ph = ps.tile([P, 512], f32, tag="h", name="hfc")
                for ci, lt in enumerate(lhsTs):
                    nc.tensor.matmul(
                        ph[:, :dout], lhsT=lt[:],
                        rhs=wsb[Wn][:, ci * dout : (ci + 1) * dout],
                        start=(ci == 0), stop=False,
                    )
                for j, (lrow, rr_) in enumerate(extra):
                    nc.tensor.matmul(
                        ph[:, :dout], lhsT=lrow, rhs=rr_,
                        start=False, stop=(j == len(extra) - 1),
                    )
                return ph

            # ----------------------------------------------- pooling (1 pass)
            def pooling(rep):
                pw = [ps_acc.tile([P, 512], f32, tag="aggA",
                                  name=f"pw{w}_{rep}") for w in range(WWIN)]
                pe = [ps_acc.tile([P, 512], f32, tag="aggB",
                                  name=f"pe{w}_{rep}") for w in range(WWIN)]
                z1 = sb.tile([1, 512], bf16, tag="zrow")
                nc.vector.memset(z1[:], 0.0)
                for w in range(WWIN):
                    nc.tensor.matmul(pw[w][:, :512], lhsT=z1[:, :P],
                                     rhs=z1[:, :512], start=True, stop=False)
                    nc.tensor.matmul(pe[w][:, :1], lhsT=z1[:, :P],
                                     rhs=z1[:, :1], start=True, stop=False)
                for g4 in range(_ceil(NB, 4)):
                    u5g = sb.tile([P, 4, 512], bf16, tag="u5ld")
                    nb4 = min(4, NB - g4 * 4)
                    nc.sync.dma_start(
                        out=u5g[:, :nb4, :],
                        in_=u5[g4 * 4 * P : (g4 * 4 + nb4) * P, :].rearrange(
                            "(a p) w -> p a w", p=P
                        ),
                    )
                    g4pairs = [
                        jw for a_ in range(nb4)
                        for jw in pairs_by_block.get(g4 * 4 + a_, [])
                    ]
                    if not g4pairs:
                        continue
                    jmin = min(j for (j, _w) in g4pairs)
                    jnum = max(j for (j, _w) in g4pairs) - jmin + 1
                    assert jnum <= 8, f"G window too wide: {jnum}"
                    Gg = sb.tile([P, 8 * P], bf16, tag="Gg")
                    nc.sync.dma_start(
                        out=Gg[:, : jnum * P],
                        in_=G_ext[:, jmin * P : (jmin + jnum) * P],
                    )
                    for a in range(nb4):
                        b = g4 * 4 + a
                        plist = pairs_by_block.get(b, [])
                        if not plist:
                            continue
                        ub = u5g[:, a, :]
                        gm = sb.tile([P, 512], f32, tag="gatem")
                        nc.vector.tensor_tensor(
                            out=gm[:], in0=ub, in1=wsb["wgrep"][:, :],
                            op=mybir.AluOpType.mult,
                        )
                        gate = sb.tile([P, 1], f32, tag="gate")
                        nc.vector.reduce_sum(
                            out=gate[:], in_=gm[:], axis=mybir.AxisListType.X
                        )
                        e = sb.tile([P, 1], f32, tag="ecol")
                        nc.scalar.activation(
                            out=e[:], in_=gate[:],
                            func=mybir.ActivationFunctionType.Exp,
                            bias=bgrep_sb[:, :], scale=1.0,
                        )
                        e_bf = sb.tile([P, 1], bf16, tag="ebf")
                        nc.vector.tensor_copy(out=e_bf[:], in_=e[:])
                        rhs512 = sb.tile([P, 512], bf16, tag="rhs512")
                        nc.vector.tensor_scalar_mul(
                            out=rhs512[:], in0=ub, scalar1=e[:, 0:1]
                        )
                        for (j, w) in plist:
                            nc.tensor.matmul(
                                pw[w][:, :512],
                                lhsT=Gg[:, (j - jmin) * P : (j - jmin + 1) * P],
                                rhs=rhs512[:], start=False, stop=False,
                            )
                            nc.tensor.matmul(
                                pe[w][:, :1],
                                lhsT=Gg[:, (j - jmin) * P : (j - jmin + 1) * P],
                                rhs=e_bf[:], start=False, stop=False,
                            )
                for w in range(WWIN):
                    nc.tensor.matmul(pw[w][:, :512], lhsT=z1[:, :P],
                                     rhs=z1[:, :512], start=False, stop=True)
                    nc.tensor.matmul(pe[w][:, :1], lhsT=z1[:, :P],
                                     rhs=z1[:, :1], start=False, stop=True)

                    pooled = sb.tile([P, 512], f32, tag="pooled")
                    nc.vector.tensor_copy(out=pooled[:], in_=pw[w][:, :512])
                    se = sb.tile([P, 1], f32, tag="se")
                    nc.vector.tensor_tensor(
                        out=se[:], in0=pe[w][:, :1], in1=eps_col[:],
                        op=mybir.AluOpType.max,
                    )
                    si = sb.tile([P, 1], f32, tag="si")
                    nc.vector.reciprocal(out=si[:], in_=se[:])
                    fcin = sb.tile([P, 512], bf16, tag="fcin")
                    nc.vector.tensor_scalar_mul(
                        out=fcin[:], in0=pooled[:], scalar1=si[:, 0:1]
                    )
                    l1 = main_matmul(
                        transpose_chunks(fcin, 512), "Wf1", 256,
                        [(ones_row[:, :], wsb["cf1"][:, :256])],
                    )
                    h1 = lrelu(l1, 256, "fc1")
                    l2 = main_matmul(
                        transpose_chunks(h1, 256), "Wf2", 128,
                        [(ones_row[:, :], wsb["cf2"][:, :128])],
                    )
                    h2 = lrelu(l2, 128, "fc2")
                    l3 = main_matmul(
                        transpose_chunks(h2, 128), "Wf3", 1,
                        [(ones_row[:, :], wsb["cf3"][:, :1])],
                    )
                    oc = sb.tile([P, 1], f32, tag="oc")
                    nc.vector.tensor_copy(out=oc[:], in_=l3[:, :1])
                    nc.sync.dma_start(
                        out=out_ext[w * P : (w + 1) * P, :], in_=oc[:]
                    )

            # ----------------- the program -----------------
            for rep in range(REPS):
                pool0build()
                conv1()
                if PHASES >= 2:
                    conv_postA(u1T, 4, "W2", 256, "c2", h2s, "L2")
                    sendbuild(h2s, sendA, 256)
                    a2a(sendA, pool2)
                    conv_postB_T(pool2, h2s, 256, "b2", u2T, "L2")
                if PHASES >= 3:
                    conv_postA(u2T, 2, "W3", 128, "c3", h3s, "L3")
                    sendbuild(h3s, sendB, 128)
                    a2a(sendB, pool3)
                    conv_postB_N(pool3, h3s, 128, "b3", u3, "L3")
                if PHASES >= 4:
                    sendbuild(u3, sendB, 128)
                    a2a(sendB, poolu3)
                    conv_pre(poolu3, u3, 128, "W4", 256, "c4r", "c4",
                             u4, "L4")
                    sendbuild(u4, sendA, 256)
                    a2a(sendA, poolu4)
                    conv_pre(poolu4, u4, 256, "W5", 512, "c5r", "c5",
                             u5, "L5")
                if PHASES >= 5:
                    pooling(rep)

    nc.finalize()
    return nc


# ----------------------------------------------------------------- frontend

_CACHE = {}


def _prepare(inputs, B):
    x = np.asarray(inputs["x"], np.float32)
    ei = np.asarray(inputs["edge_index"], np.int64)
    ea = np.asarray(inputs["edge_attr"], np.float32)
    bt = np.asarray(inputs["batch"], np.int64)
    key = hash((x.shape, ei.tobytes(), bt.tobytes(), B))
    if key not in _CACHE:
        meta, per_core, x0p = _preprocess(x, ei, ea, bt, B)
        nc = build_program(meta)
        _CACHE.clear()
        _CACHE[key] = (meta, per_core, x0p, nc)
    return _CACHE[key]


def _in_maps(meta, per_core, x0p, wf):
    maps = []
    for c in range(NCORES):
        m = dict(x0=x0p, bgrep=wf["bgrep"], **{
            n: wf[n] for n, _ in WSHAPES
        })
        m["S2"] = per_core[c]["S2"]
        m["gidx2"] = per_core[c]["gidx2"]
        m["sidx"] = per_core[c]["sidx"]
        m["sidx0"] = per_core[c]["sidx0"]
        m["x0loc"] = per_core[c]["x0loc"]
        m["rrow"] = per_core[c]["rrow"]
        m["G"] = per_core[c]["G"]
        m["D"] = per_core[c]["D"]
        maps.append(m)
    return maps


def _assemble(meta, results, inputs, B):
    GPC, WWIN = meta["GPC"], meta["WWIN"]
    out = np.empty(B, np.float32)
    for c in range(NCORES):
        out[c * GPC : (c + 1) * GPC] = results[c]["out"][:GPC, 0]
    cnt = np.bincount(np.asarray(inputs["batch"], np.int64), minlength=B)
    if (cnt == 0).any():
        Wf1, bf1 = np.asarray(inputs["Wf1"]), np.asarray(inputs["bf1"])
        Wf2, bf2 = np.asarray(inputs["Wf2"]), np.asarray(inputs["bf2"])
        Wf3, bf3 = np.asarray(inputs["Wf3"]), np.asarray(inputs["bf3"])
        lr = lambda z: np.where(z >= 0, z, 0.01 * z)
        h = lr(np.zeros(Wf1.shape[0]) @ Wf1 + bf1)
        h = lr(h @ Wf2 + bf2)
        out[cnt == 0] = float(h @ Wf3 + bf3)
    return out


def kernel(_B=B_DEFAULT, **inputs):
    meta, per_core, x0p, nc = _prepare(inputs, _B)
    wf = _fold_weights(inputs)
    maps = _in_maps(meta, per_core, x0p, wf)
    res = run_bass_kernel_spmd(nc, maps, core_ids=list(range(NCORES)))
    return _assemble(meta, res.results, inputs, _B)


# revision 26
# speedup vs baseline: 3.8640x; 1.6483x over previous
"""GCNNet forward on 8 Trainium2 NeuronCores (Bass/Tile SPMD), v2.

Strategy
--------
- Nodes partitioned graph-aligned across 8 cores (B/8 graphs per core).
- Edge aggregation (GCN symmetric norm) via one-hot matmuls in SWAPPED
  orientation: accT[feat, dst] += gathered[slot, feat]^T-free @ S[slot, dst].
  The PSUM result is directly the lhsT for the next dense matmul, so the
  conv stack needs NO PE transposes.
- Self-loops are NOT gathered: per dst block, lhsT = contiguous local
  feature block, rhs = diag(dinv^2) tile (D). Removes ~30% of gather rows.
- L1 gathers x at its true width 64.
- u1/u2 are stored transposed ([feat, node]) so the following matmul-first
  layers (L2/L3 postA) read lhsT chunks directly. h2/h3/u3/u4 stay
  node-major for AllGather + gather.
- BatchNorm affine + biases folded into weights host-side (rank-1 rows
  with rrow[n] = row-sum of norm incl self).
- Cross-core exchange: 4 bf16 AllGathers (h2, h3, u3, u4).
- Attention pooling: single pass over u5; per-graph one-hot matmuls into
  per-window PSUM accumulators, only (block, window) pairs that are
  nonzero on some core. Softmax without max-subtraction; host fixes
  empty graphs.
"""
import os
import sys

for _p in ("/opt/trn_rl_repo", "/root/.axon_site/_ro/trn_rl_repo"):
    if os.path.isdir(_p) and _p not in sys.path:
        sys.path.insert(0, _p)

import numpy as np
import ml_dtypes

import concourse.bass as bass
import concourse.bacc as bacc
import concourse.mybir as mybir
import concourse.tile as tile
from concourse.bass_utils import run_bass_kernel_spmd
from concourse.masks import make_identity

P = 128
NCORES = 8
NRANGE = 4
GBLK = 1  # blocks per gather group (HW dma_gather breaks with large chunks)
AGRP = 2  # dst blocks per agg dma_gather (>2 crashes HW: chunk too large)

bf16 = mybir.dt.float16  # working dtype (fp16: 10-bit mantissa, safe ranges)
f32 = mybir.dt.float32
i16 = mybir.dt.int16
BF = np.float16

B_DEFAULT = 2048
PHASES = 5  # debug: how many phases of the program to emit
REPS = 1  # timing: repeat the whole body REPS times inside the program
MAXG = 10**9


def set_f32_debug():
    global bf16, BF
    bf16 = mybir.dt.float32
    BF = np.float32


def set_f16():
    global bf16, BF
    bf16 = mybir.dt.float16
    BF = np.float16


def _ceil(a, b):
    return -(-a // b)


# ----------------------------------------------------------------- host prep

def _preprocess(x, edge_index, edge_attr, batch, B):
    N = x.shape[0]
    GPC = B // NCORES
    src = np.asarray(edge_index[0], np.int64)
    dst = np.asarray(edge_index[1], np.int64)
    ew = np.asarray(edge_attr, np.float64)
    batch = np.asarray(batch, np.int64)

    gstarts = np.searchsorted(batch, np.arange(0, B + 1, GPC))
    node_start = gstarts[:-1]
    node_cnt = np.diff(gstarts)
    Np = int(_ceil(max(int(node_cnt.max()), 1), P) * P)
    assert 2 * Np <= 32767, f"Np={Np} too large for int16 gather ranges"
    NB = Np // P
    RSZ = 2 * Np

    core_of = batch // GPC
    pid = core_of * Np + (np.arange(N) - node_start[core_of])
    local_graph = batch - core_of * GPC

    deg = np.bincount(dst, weights=ew, minlength=N) + 1.0
    dinv = 1.0 / np.sqrt(deg)
    dinv2 = dinv * dinv
    norm_e = dinv[src] * ew * dinv[dst]
    rvec = np.bincount(dst, weights=norm_e, minlength=N) + dinv2

    # edges only (self-loops handled via D diag tiles)
    es, ed, en = src, dst, norm_e

    e_core = core_of[ed]
    e_block = (pid[ed] % Np) // P
    e_dl = pid[ed] % P

    WWIN = _ceil(GPC, P)

    # ---- selective exchange (AllToAll): per (sender, receiver) row lists
    s_core_e = core_of[es]
    l_src = pid[es] % Np  # sender-local row id
    sendlists = [[None] * NCORES for _ in range(NCORES)]
    maxlen = 1
    for s in range(NCORES):
        for r in range(NCORES):
            mm = (s_core_e == s) & (e_core == r)
            u = np.unique(l_src[mm])
            sendlists[s][r] = u
            maxlen = max(maxlen, len(u))
    CH = _ceil(maxlen, 64) * 64  # 2*CH must be a multiple of 128
    NSEND = NCORES * CH
    NSEND = _ceil(NSEND, P) * P
    assert NSEND <= 32767, f"NSEND={NSEND} exceeds int16 gather range"
    NST = NSEND // P  # send tiles

    # pool row for each edge: s*CH + position in sendlists[s][r]
    pool_row = np.empty(len(es), np.int64)
    for s in range(NCORES):
        for r in range(NCORES):
            mm = (s_core_e == s) & (e_core == r)
            if mm.any():
                pool_row[mm] = s * CH + np.searchsorted(
                    sendlists[s][r], l_src[mm]
                )

    # single-range bucketing for the 4 exchanged layers
    key2 = (e_core * NB + e_block).astype(np.int64)
    cnt2 = np.bincount(key2, minlength=NCORES * NB).reshape(NCORES, NB)
    K2 = _ceil(cnt2.max(axis=0), P)  # [NB]
    tile_of_b2 = np.concatenate(([0], np.cumsum(K2)))[:-1]
    ntilesE = int(K2.sum())
    order2 = np.lexsort((e_block, e_core))
    excl2 = np.concatenate(
        ([0], np.cumsum(np.bincount(key2, minlength=NCORES * NB)))
    )
    pos2 = np.arange(len(order2)) - excl2[key2[order2]]
    slot2 = tile_of_b2[e_block[order2]] * P + pos2

    # pooling (block, window) pairs: union over cores of nonzero G tiles
    lg_all = []
    pair_set = set()
    for c in range(NCORES):
        lg = np.full(Np, -1, np.int64)
        ncnt = int(node_cnt[c])
        lg[:ncnt] = local_graph[node_start[c] : node_start[c] + ncnt]
        lg_all.append(lg)
        for b in range(NB):
            seg = lg[b * P : (b + 1) * P]
            for w in np.unique(seg[seg >= 0] // P):
                pair_set.add((b, int(w)))
    pairs = sorted(pair_set)
    npairs = len(pairs)
    pairs_by_block = {}
    for j, (b, w) in enumerate(pairs):
        pairs_by_block.setdefault(b, []).append((j, w))

    G_all, D_all = [], []
    S2_all, idx2_all, sidx_all, sidx0_all = [], [], [], []
    rrow = np.zeros((NCORES, Np), np.float32)
    for c in range(NCORES):
        # exchanged-layer S2/gidx2 (single range into the A2A pool)
        S2 = np.zeros((ntilesE, P, P), np.float32)
        idx2_lin = np.zeros(ntilesE * P, np.int16)
        m2 = e_core[order2] == c
        sl2 = slot2[m2]
        S2[sl2 // P, sl2 % P, e_dl[order2][m2]] = en[order2][m2]
        idx2_lin[sl2] = pool_row[order2][m2].astype(np.int16)
        S2_all.append(
            np.ascontiguousarray(S2.transpose(1, 0, 2))
            .reshape(P, ntilesE * P).astype(BF)
        )
        p2 = idx2_lin.reshape(-1, 16).T
        idx2_all.append(np.tile(p2, (8, 1)))

        sidx_lin = np.zeros(NSEND, np.int16)
        for r in range(NCORES):
            L = sendlists[c][r]
            sidx_lin[r * CH : r * CH + len(L)] = L.astype(np.int16)
        ps_ = sidx_lin.reshape(-1, 16).T
        sidx_all.append(np.tile(ps_, (8, 1)))

        # pool0 build indices: row s*CH+i <- x0[s*Np + sendlists[s][c][i]],
        # gathered per range r'=s//2 from x0[r'*RSZ:(r'+1)*RSZ]
        sidx0_lin = np.zeros(NSEND, np.int16)
        for s in range(NCORES):
            L = sendlists[s][c]
            sidx0_lin[s * CH : s * CH + len(L)] = (
                (s % 2) * Np + L
            ).astype(np.int16)
        p0_ = sidx0_lin.reshape(-1, 16).T
        sidx0_all.append(np.tile(p0_, (8, 1)))

        ncnt = int(node_cnt[c])
        rrow[c, :ncnt] = rvec[node_start[c] : node_start[c] + ncnt]

        D = np.zeros((NB, P, P), np.float32)
        dloc = dinv2[node_start[c] : node_start[c] + ncnt]
        nn = np.arange(ncnt)
        D[nn // P, nn % P, nn % P] = dloc
        D_all.append(
            np.ascontiguousarray(D.transpose(1, 0, 2))
            .reshape(P, NB * P).astype(BF)
        )

        G = np.zeros((npairs, P, P), np.float32)
        lg = lg_all[c]
        for j, (b, w) in enumerate(pairs):
            seg = lg[b * P : (b + 1) * P]
            v = (seg >= 0) & (seg // P == w)
            G[j, np.nonzero(v)[0], seg[v] - w * P] = 1.0
        G_all.append(
            np.ascontiguousarray(G.transpose(1, 0, 2))
            .reshape(P, npairs * P).astype(BF)
        )

    IN = x.shape[1]
    # dma_gather requires elem_size to be a multiple of 256B -> pad x to 128
    x0p = np.zeros((NCORES * Np, P), np.float32)
    x0p[pid, :IN] = np.asarray(x, np.float32)
    x0p = x0p.astype(BF)
    x0loc = [x0p[c * Np : (c + 1) * Np] for c in range(NCORES)]

    meta = dict(
        N=N, B=B, GPC=GPC, Np=Np, NB=NB, RSZ=RSZ, WWIN=WWIN,
        node_start=node_start, node_cnt=node_cnt,
        IN=IN, npairs=npairs, pairs=pairs, pairs_by_block=pairs_by_block,
        CH=CH, NSEND=NSEND, NST=NST, K2=K2, tile_of_b2=tile_of_b2,
        ntilesE=ntilesE,
    )
    per_core = [
        dict(x0loc=x0loc[c],
             rrow=rrow[c].astype(BF)[None, :], G=G_all[c], D=D_all[c],
             S2=S2_all[c], gidx2=idx2_all[c], sidx=sidx_all[c],
             sidx0=sidx0_all[c])
        for c in range(NCORES)
    ]
    return meta, per_core, x0p


def _fold_weights(inp):
    f = lambda k: np.asarray(inp[k], np.float64)
    A, Bb = [], []
    for i in range(1, 6):
        a = f("g%d" % i) / np.sqrt(f("v%d" % i) + 1e-5)
        A.append(a)
        Bb.append(f("be%d" % i) - f("m%d" % i) * a)

    def pack(W):
        din, dout = W.shape
        nch = _ceil(din, P)
        Wp = np.zeros((nch * P, dout))
        Wp[:din] = W
        return (
            np.ascontiguousarray(Wp.reshape(nch, P, dout).transpose(1, 0, 2))
            .reshape(P, nch * dout).astype(BF)
        )

    o = {}
    o["W1"] = pack(f("W1"))
    o["c1"] = f("b1")[None, :].astype(BF)
    o["W2"] = pack(A[0][:, None] * f("W2"))
    o["c2"] = (Bb[0] @ f("W2"))[None, :].astype(BF)
    o["b2"] = f("b2")[None, :].astype(BF)
    o["W3"] = pack(A[1][:, None] * f("W3"))
    o["c3"] = (Bb[1] @ f("W3"))[None, :].astype(BF)
    o["b3"] = f("b3")[None, :].astype(BF)
    o["W4"] = pack(A[2][:, None] * f("W4"))
    o["c4r"] = (Bb[2] @ f("W4"))[None, :].astype(BF)
    o["c4"] = f("b4")[None, :].astype(BF)
    o["W5"] = pack(A[3][:, None] * f("W5"))
    o["c5r"] = (Bb[3] @ f("W5"))[None, :].astype(BF)
    o["c5"] = f("b5")[None, :].astype(BF)
    wg = A[4] * f("Wg")[:, 0]
    o["wgrep"] = np.tile(wg[None, :], (P, 1)).astype(BF)
    o["bgrep"] = np.full(
        (P, 1), float(Bb[4] @ f("Wg")[:, 0] + f("bg")[0]), np.float32
    )
    o["Wf1"] = pack(A[4][:, None] * f("Wf1"))
    o["cf1"] = (f("bf1") + Bb[4] @ f("Wf1"))[None, :].astype(BF)
    o["Wf2"] = pack(f("Wf2"))
    o["cf2"] = f("bf2")[None, :].astype(BF)
    o["Wf3"] = pack(f("Wf3"))
    o["cf3"] = f("bf3")[None, :].astype(BF)
    return o


WSHAPES = [
    ("W1", [P, 512]), ("c1", [1, 512]),
    ("W2", [P, 4 * 256]), ("c2", [1, 256]), ("b2", [1, 256]),
    ("W3", [P, 2 * 128]), ("c3", [1, 128]), ("b3", [1, 128]),
    ("W4", [P, 256]), ("c4r", [1, 256]), ("c4", [1, 256]),
    ("W5", [P, 2 * 512]), ("c5r", [1, 512]), ("c5", [1, 512]),
    ("wgrep", [P, 512]), ("Wf1", [P, 4 * 256]), ("cf1", [1, 256]),
    ("Wf2", [P, 2 * 128]), ("cf2", [1, 128]),
    ("Wf3", [P, 1]), ("cf3", [1, 1]),
]


# ------------------------------------------------------------- device build

def build_program(meta):
    Np, NB, RSZ = meta["Np"], meta["NB"], meta["RSZ"]
    WWIN, IN = meta["WWIN"], meta["IN"]
    npairs, pairs_by_block = meta["npairs"], meta["pairs_by_block"]
    NSEND, NST = meta["NSEND"], meta["NST"]
    K2, tile_of_b2, ntilesE = meta["K2"], meta["tile_of_b2"], meta["ntilesE"]
    NPT = NCORES * Np

    nc = bacc.Bacc(None)
    dp = nc.declare_dram_parameter
    x0_ext = dp("x0", [NPT, P], bf16, isOutput=False)
    x0loc_ext = dp("x0loc", [Np, P], bf16, isOutput=False)
    S2_ext = dp("S2", [P, ntilesE * P], bf16, isOutput=False)
    gidx2_ext = dp("gidx2", [P, ntilesE * 8], i16, isOutput=False)
    sidx_ext = dp("sidx", [P, NST * 8], i16, isOutput=False)
    sidx0_ext = dp("sidx0", [P, NST * 8], i16, isOutput=False)
    rrow_ext = dp("rrow", [1, Np], bf16, isOutput=False)
    G_ext = dp("G", [P, npairs * P], bf16, isOutput=False)
    D_ext = dp("D", [P, NB * P], bf16, isOutput=False)
    wext = {n: dp(n, sh, bf16, isOutput=False) for n, sh in WSHAPES}
    bgrep_ext = dp("bgrep", [P, 1], f32, isOutput=False)
    out_ext = dp("out", [WWIN * P, 1], f32, isOutput=True)

    u1T = nc.dram_tensor("u1T", [4 * P, Np], bf16)
    u2T = nc.dram_tensor("u2T", [2 * P, Np], bf16)
    h2s = nc.dram_tensor("h2s", [Np, 256], bf16)
    h3s = nc.dram_tensor("h3s", [Np, 128], bf16)
    u3 = nc.dram_tensor("u3", [Np, 128], bf16)
    u4 = nc.dram_tensor("u4", [Np, 256], bf16)
    u5 = nc.dram_tensor("u5", [Np, 512], bf16)
    pool0 = nc.dram_tensor("pool0", [NSEND, P], bf16)
    sendA = nc.dram_tensor("sendA", [NSEND, 256], bf16)
    sendB = nc.dram_tensor("sendB", [NSEND, 128], bf16)
    pool2 = nc.dram_tensor("pool2", [NSEND, 256], bf16)
    pool3 = nc.dram_tensor("pool3", [NSEND, 128], bf16)
    poolu3 = nc.dram_tensor("poolu3", [NSEND, 128], bf16)
    poolu4 = nc.dram_tensor("poolu4", [NSEND, 256], bf16)
    RG = [list(range(NCORES))]

    with tile.TileContext(nc) as tc:
        with (
            tc.tile_pool(name="persist", bufs=1) as pp,
            tc.tile_pool(name="sb", bufs=2) as sb,
            tc.tile_pool(name="sb3", bufs=3) as sb3,
            tc.tile_pool(name="ps", bufs=2, space="PSUM") as ps,
            tc.tile_pool(name="ps_acc", bufs=2, space="PSUM") as ps_acc,
        ):
            gidx2_sb = pp.tile([P, ntilesE * 8], i16)
            nc.sync.dma_start(out=gidx2_sb[:], in_=gidx2_ext[:, :])
            sidx_sb = pp.tile([P, NST * 8], i16)
            nc.sync.dma_start(out=sidx_sb[:], in_=sidx_ext[:, :])
            sidx0_sb = pp.tile([P, NST * 8], i16)
            nc.sync.dma_start(out=sidx0_sb[:], in_=sidx0_ext[:, :])
            wsb = {}
            for n, sh in WSHAPES:
                wsb[n] = pp.tile(sh, bf16, tag="w_" + n, name="w_" + n)
                nc.sync.dma_start(out=wsb[n][:], in_=wext[n][:, :])
            bgrep_sb = pp.tile([P, 1], f32)
            nc.sync.dma_start(out=bgrep_sb[:], in_=bgrep_ext[:, :])
            D_sb = pp.tile([P, NB * P], bf16, name="D_sb")
            nc.sync.dma_start(out=D_sb[:], in_=D_ext[:, :])
            rrow_sb = pp.tile([1, Np], bf16, name="rrow_sb")
            nc.sync.dma_start(out=rrow_sb[:], in_=rrow_ext[:, :])
            ident = pp.tile([P, P], bf16)
            make_identity(nc, ident[:])
            ones_row = pp.tile([1, P], bf16)
            nc.vector.memset(ones_row[:], 1.0)
            eps_col = pp.tile([P, 1], f32)
            nc.vector.memset(eps_col[:], 1e-20)

            def pool0build():
                """local build of the L1 pool from the replicated x0:
                sender s's chunk lies in range s//2 of x0."""
                GW = 8
                ntr = NST // 4  # tiles per range (2*CH rows)
                for rp in range(4):
                    base = rp * ntr
                    for gg in range(_ceil(ntr, GW)):
                        nt = min(GW, ntr - gg * GW)
                        t = sb.tile([P, GW, P], bf16, tag="p0b",
                                    name=f"p0b_{rp}_{gg}")
                        nc.gpsimd.dma_gather(
                            out_ap=t[:, :nt, :],
                            in_ap=x0_ext[rp * RSZ : (rp + 1) * RSZ, :],
                            idxs_ap=sidx0_sb[
                                :, (base + gg * GW) * 8 : (base + gg * GW + nt) * 8
                            ],
                            num_idxs=nt * P,
                            num_idxs_reg=nt * P,
                            elem_size=P,
                        )
                        nc.sync.dma_start(
                            out=pool0[
                                (base + gg * GW) * P : (base + gg * GW + nt) * P, :
                            ].rearrange("(a p) w -> p a w", p=P),
                            in_=t[:, :nt, :],
                        )

            NTG4 = max(
                sum(int(K2[b]) for b in range(g0, min(g0 + AGRP, NB)))
                for g0 in range(0, NB, AGRP)
            )

            def gather_pool_grp(b0, pool, w):
                """one gather + S load covering blocks b0..b0+AGRP-1."""
                b1 = min(b0 + AGRP, NB)
                t0 = int(tile_of_b2[b0])
                nt = sum(int(K2[b]) for b in range(b0, b1))
                gall = sb.tile([P, NTG4, w], bf16, tag=f"gpw{w}",
                               name=f"gp_{b0}")
                nc.gpsimd.dma_gather(
                    out_ap=gall[:, :nt, :],
                    in_ap=pool[0:NSEND, :w],
                    idxs_ap=gidx2_sb[:, t0 * 8 : (t0 + nt) * 8],
                    num_idxs=nt * P,
                    num_idxs_reg=nt * P,
                    elem_size=w,
                )
                s_grp = sb.tile([P, NTG4 * P], bf16, tag="sgrp2",
                                name=f"sg2_{b0}")
                nc.sync.dma_start(
                    out=s_grp[:, : nt * P],
                    in_=S2_ext[:, t0 * P : (t0 + nt) * P],
                )
                return (gall, s_grp, t0)

            def gather_for(cache, b, pool, w):
                b0 = (b // AGRP) * AGRP
                if cache.get("b0") != b0:
                    cache["b0"] = b0
                    cache["gts"] = gather_pool_grp(b0, pool, w)
                return block_tiles(cache["gts"], b)

            def block_tiles(gts, b):
                gall, s_grp, g_t0 = gts
                t0 = int(tile_of_b2[b])
                return (gall, s_grp, g_t0,
                        [t0 + k for k in range(int(K2[b]))])

            def sendbuild(src_loc, send_dram, w):
                """gather own rows every peer needs into the A2A send buf."""
                GW = 8
                for gg in range(_ceil(NST, GW)):
                    nt = min(GW, NST - gg * GW)
                    t = sb.tile([P, GW, w], bf16, tag=f"snd{w}",
                                name=f"snd{w}_{gg}")
                    nc.gpsimd.dma_gather(
                        out_ap=t[:, :nt, :],
                        in_ap=src_loc[0:Np, :w],
                        idxs_ap=sidx_sb[:, gg * GW * 8 : (gg * GW + nt) * 8],
                        num_idxs=nt * P,
                        num_idxs_reg=nt * P,
                        elem_size=w,
                    )
                    nc.sync.dma_start(
                        out=send_dram[gg * GW * P : (gg * GW + nt) * P, :]
                        .rearrange("(a p) w -> p a w", p=P),
                        in_=t[:, :nt, :],
                    )

            def a2a(src, dst):
                nc.gpsimd.collective_compute(
                    "AllToAll", mybir.AluOpType.bypass,
                    replica_groups=RG, ins=[src[:]], outs=[dst[:]],
                )

            def load_self4(src_loc, g4, w, tag):
                """contiguous rows for blocks 4*g4 .. 4*g4+3 -> [P, 4, w]."""
                nb4 = min(4, NB - g4 * 4)
                t = sb.tile([P, 4, w], bf16, tag=tag)
                nc.sync.dma_start(
                    out=t[:, :nb4, :],
                    in_=src_loc[g4 * 4 * P : (g4 * 4 + nb4) * P, :w].rearrange(
                        "(a p) w -> p a w", p=P
                    ),
                )
                return t

            def seg_aggT(b, gts, w, self_sb, bias_row):
                """swapped-orientation aggregation: accT[feat, dst].

                Returns list of PSUM tiles, one per 128-chunk of w. w may be
                smaller than the gathered width (trailing cols ignored).
                self_sb: [P, >=w] node-major contiguous self rows for block b.
                bias_row: [1, w] row to add per-dst (or None).
                """
                gall, s_grp, g_t0, tiles = gts
                nchunk = _ceil(w, P)
                accs = []
                for c in range(nchunk):
                    cw = min(P, w - c * P)
                    acc = ps_acc.tile([P, 512], f32, tag=f"agg{'AB'[c]}",
                                      name=f"agg{c}")
                    first = True
                    for tg in tiles:
                        nc.tensor.matmul(
                            acc[:cw, :P],
                            lhsT=gall[:, tg - g_t0, c * P : c * P + cw],
                            rhs=s_grp[:, (tg - g_t0) * P : (tg - g_t0 + 1) * P],
                            start=first, stop=False,
                        )
                        first = False
                    last = bias_row is None
                    nc.tensor.matmul(
                        acc[:cw, :P],
                        lhsT=self_sb[:, c * P : c * P + cw],
                        rhs=D_sb[:, b * P : (b + 1) * P],
                        start=first, stop=last,
                    )
                    if bias_row is not None:
                        nc.tensor.matmul(
                            acc[:cw, :P],
                            lhsT=bias_row[:, c * P : c * P + cw],
                            rhs=ones_row[:, :],
                            start=False, stop=True,
                        )
                    accs.append((acc, cw))
                return accs

            def accs_to_sbuf(accs, tag):
                outs = []
                for i, (acc, cw) in enumerate(accs):
                    st = sb.tile([P, P], bf16, tag=f"{tag}{i}")
                    nc.vector.tensor_copy(out=st[:cw, :], in_=acc[:cw, :P])
                    outs.append((st, cw))
                return outs

            def lrelu(psum, w, tag):
                u = sb.tile([P, w], bf16, tag=f"u{tag}")
                nc.scalar.activation(
                    out=u[:], in_=psum[:, :w],
                    func=mybir.ActivationFunctionType.Lrelu, alpha=0.01,
                )
                return u

            # -------------------------------------------------- L1 conv_pre
            def conv1():
                gc1 = {}
                for g4 in range(_ceil(NB, 4)):
                    selfs = load_self4(x0loc_ext, g4, P, "self1")
                    stages = [
                        sb.tile([P, 4 * P], bf16, tag=f"u1st{oc}",
                                name=f"u1st{oc}")
                        for oc in range(4)
                    ]
                    for a in range(4):
                        b = g4 * 4 + a
                        if b >= NB:
                            break
                        gts = gather_for(gc1, b, pool0, P)
                        accs = seg_aggT(b, gts, IN, selfs[:, a, :], None)
                        aggT = accs_to_sbuf(accs, "aggT1")[0]
                        st0, cw = aggT
                        for oc in range(4):
                            phT = ps.tile([P, 512], f32, tag="h",
                                          name="phT")
                            nc.tensor.matmul(
                                phT[:, :P],
                                lhsT=wsb["W1"][:cw, oc * P : (oc + 1) * P],
                                rhs=st0[:cw, :],
                                start=True, stop=False,
                            )
                            nc.tensor.matmul(
                                phT[:, :P],
                                lhsT=wsb["c1"][:, oc * P : (oc + 1) * P],
                                rhs=ones_row[:, :],
                                start=False, stop=True,
                            )
                            nc.scalar.activation(
                                out=stages[oc][:, a * P : (a + 1) * P],
                                in_=phT[:, :P],
                                func=mybir.ActivationFunctionType.Lrelu,
                                alpha=0.01,
                            )
                    nb4 = min(4, NB - g4 * 4)
                    for oc in range(4):
                        nc.sync.dma_start(
                            out=u1T[oc * P : (oc + 1) * P,
                                    g4 * 4 * P : (g4 * 4 + nb4) * P],
                            in_=stages[oc][:, : nb4 * P],
                        )

            # ---------------------------------------- postA: h = u @ W + c
            def conv_postA(uT_src, nch, Wn, dout, cn, hdst, tag):
                for g4 in range(_ceil(NB, 4)):
                    nb4g = min(4, NB - g4 * 4)
                    ins = []
                    for ic in range(nch):
                        t = sb.tile([P, 4 * P], bf16, tag=f"pA{tag}{ic}")
                        nc.sync.dma_start(
                            out=t[:, : nb4g * P],
                            in_=uT_src[ic * P : (ic + 1) * P,
                                       g4 * 4 * P : (g4 * 4 + nb4g) * P],
                        )
                        ins.append(t)
                    hstage = sb.tile([P, 4, dout], bf16, tag=f"hst{tag}",
                                     name=f"hst{tag}")
                    for a in range(4):
                        b = g4 * 4 + a
                        if b >= NB:
                            break
                        ph = ps.tile([P, 512], f32, tag="h", name=f"h{tag}")
                        for ic in range(nch):
                            nc.tensor.matmul(
                                ph[:, :dout],
                                lhsT=ins[ic][:, a * P : (a + 1) * P],
                                rhs=wsb[Wn][:, ic * dout : (ic + 1) * dout],
                                start=(ic == 0), stop=False,
                            )
                        nc.tensor.matmul(
                            ph[:, :dout], lhsT=ones_row[:, :],
                            rhs=wsb[cn][:, :dout],
                            start=False, stop=True,
                        )
                        nc.vector.tensor_copy(
                            out=hstage[:, a, :], in_=ph[:, :dout]
                        )
                    nc.sync.dma_start(
                        out=hdst[g4 * 4 * P : (g4 * 4 + nb4g) * P, :]
                        .rearrange("(a p) w -> p a w", p=P),
                        in_=hstage[:, :nb4g, :],
                    )

            # ------------------- postB: agg(h) + b -> lrelu -> uT (swapped)
            def conv_postB_T(pool, src_loc, w, bn, uT_dst, tag):
                nchunk = w // P
                gcB = {}
                for g4 in range(_ceil(NB, 4)):
                    selfs = load_self4(src_loc, g4, w, f"selfB{tag}")
                    stages = [
                        sb.tile([P, 4 * P], bf16, tag=f"uB{tag}{c}",
                                name=f"uB{tag}{c}")
                        for c in range(nchunk)
                    ]
                    for a in range(4):
                        b = g4 * 4 + a
                        if b >= NB:
                            break
                        gts = gather_for(gcB, b, pool, w)
                        accs = seg_aggT(b, gts, w, selfs[:, a, :],
                                        wsb[bn][:, :w])
                        for c, (acc, cw) in enumerate(accs):
                            nc.scalar.activation(
                                out=stages[c][:, a * P : (a + 1) * P],
                                in_=acc[:cw, :P],
                                func=mybir.ActivationFunctionType.Lrelu,
                                alpha=0.01,
                            )
                    nb4 = min(4, NB - g4 * 4)
                    for c in range(nchunk):
                        nc.sync.dma_start(
                            out=uT_dst[c * P : (c + 1) * P,
                                       g4 * 4 * P : (g4 * 4 + nb4) * P],
                            in_=stages[c][:, : nb4 * P],
                        )

            # ------- postB normal orientation: agg(h) + b -> lrelu -> u [Np,w]
            def conv_postB_N(pool, src_loc, w, bn, u_dst, tag):
                gcN = {}
                for g4 in range(_ceil(NB, 4)):
                    selfs = load_self4(src_loc, g4, w, f"selfN{tag}")
                    stage = sb.tile([P, 4, w], bf16, tag=f"uN{tag}")
                    for a in range(4):
                        b = g4 * 4 + a
                        if b >= NB:
                            break
                        gall, s_grp, g_t0, tiles = gather_for(gcN, b, pool, w)
                        acc = ps_acc.tile([P, 512], f32, tag="aggA",
                                          name="aggN")
                        first = True
                        for tg in tiles:
                            nc.tensor.matmul(
                                acc[:, :w],
                                lhsT=s_grp[:, (tg - g_t0) * P : (tg - g_t0 + 1) * P],
                                rhs=gall[:, tg - g_t0, :],
                                start=first, stop=False,
                            )
                            first = False
                        nc.tensor.matmul(
                            acc[:, :w], lhsT=D_sb[:, b * P : (b + 1) * P],
                            rhs=selfs[:, a, :], start=first, stop=False,
                        )
                        nc.tensor.matmul(
                            acc[:, :w], lhsT=ones_row[:, :], rhs=wsb[bn][:, :w],
                            start=False, stop=True,
                        )
                        nc.scalar.activation(
                            out=stage[:, a, :], in_=acc[:, :w],
                            func=mybir.ActivationFunctionType.Lrelu,
                            alpha=0.01,
                        )
                    nb4 = min(4, NB - g4 * 4)
                    nc.sync.dma_start(
                        out=u_dst[g4 * 4 * P : (g4 * 4 + nb4) * P, :]
                        .rearrange("(a p) w -> p a w", p=P),
                        in_=stage[:, :nb4, :],
                    )

            # -------- conv_pre (L4/L5): agg(u) swapped, then main matmul
            def conv_pre(pool, src_loc, w, Wn, dout, crn, cn, u_dst, tag):
                gcP = {}
                for g4 in range(_ceil(NB, 4)):
                    selfs = load_self4(src_loc, g4, w, f"selfP{tag}")
                    stage = sb.tile([P, 4, dout], bf16, tag=f"uP{tag}")
                    for a in range(4):
                        b = g4 * 4 + a
                        if b >= NB:
                            break
                        gts = gather_for(gcP, b, pool, w)
                        accs = seg_aggT(b, gts, w, selfs[:, a, :], None)
                        aggTs = accs_to_sbuf(accs, f"agg{tag}")
                        ph = ps.tile([P, 512], f32, tag="h", name=f"hP{tag}")
                        for c, (st, cw) in enumerate(aggTs):
                            nc.tensor.matmul(
                                ph[:, :dout],
                                lhsT=st[:cw, :],
                                rhs=wsb[Wn][:cw, c * dout : (c + 1) * dout],
                                start=(c == 0), stop=False,
                            )
                        nc.tensor.matmul(
                            ph[:, :dout],
                            lhsT=rrow_sb[:, b * P : (b + 1) * P],
                            rhs=wsb[crn][:, :dout],
                            start=False, stop=False,
                        )
                        nc.tensor.matmul(
                            ph[:, :dout], lhsT=ones_row[:, :],
                            rhs=wsb[cn][:, :dout],
                            start=False, stop=True,
                        )
                        nc.scalar.activation(
                            out=stage[:, a, :], in_=ph[:, :dout],
                            func=mybir.ActivationFunctionType.Lrelu,
                            alpha=0.01,
                        )
                    nb4 = min(4, NB - g4 * 4)
                    nc.sync.dma_start(
                        out=u_dst[g4 * 4 * P : (g4 * 4 + nb4) * P, :]
                        .rearrange("(a p) w -> p a w", p=P),
                        in_=stage[:, :nb4, :],
                    )

            def transpose_chunks(src_sb, w):
                outs = []
                for ci in range(w // P):
                    pt = ps.tile([P, P], bf16, tag="trps")
                    nc.tensor.transpose(
                        out=pt[:], in_=src_sb[:, ci * P : (ci + 1) * P],
                        identity=ident[:],
                    )
                    st = sb.tile([P, P], bf16, tag="trsb")
                    nc.vector.tensor_copy(out=st[:], in_=pt[:])
                    outs.append(st)
                return outs

            def main_matmul(lhsTs, Wn, dout, extra):
                ph = ps.tile([P, 512], f32, tag="h", name="hfc")
                for ci, lt in enumerate(lhsTs):
                    nc.tensor.matmul(
                        ph[:, :dout], lhsT=lt[:],
                        rhs=wsb[Wn][:, ci * dout : (ci + 1) * dout],
                        start=(ci == 0), stop=False,
                    )
                for j, (lrow, rr_) in enumerate(extra):
                    nc.tensor.matmul(
                        ph[:, :dout], lhsT=lrow, rhs=rr_,
                        start=False, stop=(j == len(extra) - 1),
                    )
                return ph

            # ----------------------------------------------- pooling (1 pass)
            def pooling(rep):
                pw = [ps_acc.tile([P, 512], f32, tag="aggA",
                                  name=f"pw{w}_{rep}") for w in range(WWIN)]
                pe = [ps_acc.tile([P, 512], f32, tag="aggB",
                                  name=f"pe{w}_{rep}") for w in range(WWIN)]
                z1 = sb.tile([1, 512], bf16, tag="zrow")
                nc.vector.memset(z1[:], 0.0)
                for w in range(WWIN):
                    nc.tensor.matmul(pw[w][:, :512], lhsT=z1[:, :P],
                                     rhs=z1[:, :512], start=True, stop=False)
                    nc.tensor.matmul(pe[w][:, :1], lhsT=z1[:, :P],
                                     rhs=z1[:, :1], start=True, stop=False)
                for g4 in range(_ceil(NB, 4)):
                    u5g = sb.tile([P, 4, 512], bf16, tag="u5ld")
                    nb4 = min(4, NB - g4 * 4)
                    nc.sync.dma_start(
                        out=u5g[:, :nb4, :],
                        in_=u5[g4 * 4 * P : (g4 * 4 + nb4) * P, :].rearrange(
                            "(a p) w -> p a w", p=P
                        ),
                    )
                    g4pairs = [
                        jw for a_ in range(nb4)
                        for jw in pairs_by_block.get(g4 * 4 + a_, [])
                    ]
                    if not g4pairs:
                        continue
                    jmin = min(j for (j, _w) in g4pairs)
                    jnum = max(j for (j, _w) in g4pairs) - jmin + 1
                    assert jnum <= 8, f"G window too wide: {jnum}"
                    Gg = sb.tile([P, 8 * P], bf16, tag="Gg")
                    nc.sync.dma_start(
                        out=Gg[:, : jnum * P],
                        in_=G_ext[:, jmin * P : (jmin + jnum) * P],
                    )
                    for a in range(nb4):
                        b = g4 * 4 + a
                        plist = pairs_by_block.get(b, [])
                        if not plist:
                            continue
                        ub = u5g[:, a, :]
                        gm = sb.tile([P, 512], f32, tag="gatem")
                        nc.vector.tensor_tensor(
                            out=gm[:], in0=ub, in1=wsb["wgrep"][:, :],
                            op=mybir.AluOpType.mult,
                        )
                        gate = sb.tile([P, 1], f32, tag="gate")
                        nc.vector.reduce_sum(
                            out=gate[:], in_=gm[:], axis=mybir.AxisListType.X
                        )
                        e = sb.tile([P, 1], f32, tag="ecol")
                        nc.scalar.activation(
                            out=e[:], in_=gate[:],
                            func=mybir.ActivationFunctionType.Exp,
                            bias=bgrep_sb[:, :], scale=1.0,
                        )
                        e_bf = sb.tile([P, 1], bf16, tag="ebf")
                        nc.vector.tensor_copy(out=e_bf[:], in_=e[:])
                        rhs512 = sb.tile([P, 512], bf16, tag="rhs512")
                        nc.vector.tensor_scalar_mul(
                            out=rhs512[:], in0=ub, scalar1=e[:, 0:1]
                        )
                        for (j, w) in plist:
                            nc.tensor.matmul(
                                pw[w][:, :512],
                                lhsT=Gg[:, (j - jmin) * P : (j - jmin + 1) * P],
                                rhs=rhs512[:], start=False, stop=False,
                            )
                            nc.tensor.matmul(
                                pe[w][:, :1],
                                lhsT=Gg[:, (j - jmin) * P : (j - jmin + 1) * P],
                                rhs=e_bf[:], start=False, stop=False,
                            )
                for w in range(WWIN):
                    nc.tensor.matmul(pw[w][:, :512], lhsT=z1[:, :P],
                                     rhs=z1[:, :512], start=False, stop=True)
                    nc.tensor.matmul(pe[w][:, :1], lhsT=z1[:, :P],
                                     rhs=z1[:, :1], start=False, stop=True)

                    pooled = sb.tile([P, 512], f32, tag="pooled")
                    nc.vector.tensor_copy(out=pooled[:], in_=pw[w][:, :512])
                    se = sb.tile([P, 1], f32, tag="se")
                    nc.vector.tensor_tensor(
                        out=se[:], in0=pe[w][:, :1], in1=eps_col[:],
                        op=mybir.AluOpType.max,
                    )
                    si = sb.tile([P, 1], f32, tag="si")
                    nc.vector.reciprocal(out=si[:], in_=se[:])
                    fcin = sb.tile([P, 512], bf16, tag="fcin")
                    nc.vector.tensor_scalar_mul(
                        out=fcin[:], in0=pooled[:], scalar1=si[:, 0:1]
                    )
                    l1 = main_matmul(
                        transpose_chunks(fcin, 512), "Wf1", 256,
                        [(ones_row[:, :], wsb["cf1"][:, :256])],
                    )
                    h1 = lrelu(l1, 256, "fc1")
                    l2 = main_matmul(
                        transpose_chunks(h1, 256), "Wf2", 128,
                        [(ones_row[:, :], wsb["cf2"][:, :128])],
                    )
                    h2 = lrelu(l2, 128, "fc2")
                    l3 = main_matmul(
                        transpose_chunks(h2, 128), "Wf3", 1,
                        [(ones_row[:, :], wsb["cf3"][:, :1])],
                    )
                    oc = sb.tile([P, 1], f32, tag="oc")
                    nc.vector.tensor_copy(out=oc[:], in_=l3[:, :1])
                    nc.sync.dma_start(
                        out=out_ext[w * P : (w + 1) * P, :], in_=oc[:]
                    )

            # ----------------- the program -----------------
            for rep in range(REPS):
                pool0build()
                conv1()
                if PHASES >= 2:
                    conv_postA(u1T, 4, "W2", 256, "c2", h2s, "L2")
                    sendbuild(h2s, sendA, 256)
                    a2a(sendA, pool2)
                    conv_postB_T(pool2, h2s, 256, "b2", u2T, "L2")
                if PHASES >= 3:
                    conv_postA(u2T, 2, "W3", 128, "c3", h3s, "L3")
                    sendbuild(h3s, sendB, 128)
                    a2a(sendB, pool3)
                    conv_postB_N(pool3, h3s, 128, "b3", u3, "L3")
                if PHASES >= 4:
                    sendbuild(u3, sendB, 128)
                    a2a(sendB, poolu3)
                    conv_pre(poolu3, u3, 128, "W4", 256, "c4r", "c4",
                             u4, "L4")
                    sendbuild(u4, sendA, 256)
                    a2a(sendA, poolu4)
                    conv_pre(poolu4, u4, 256, "W5", 512, "c5r", "c5",
                             u5, "L5")
                if PHASES >= 5:
                    pooling(rep)

    nc.finalize()
    return nc


# ----------------------------------------------------------------- frontend

_CACHE = {}


def _prepare(inputs, B):
    x = np.asarray(inputs["x"], np.float32)
    ei = np.asarray(inputs["edge_index"], np.int64)
    ea = np.asarray(inputs["edge_attr"], np.float32)
    bt = np.asarray(inputs["batch"], np.int64)
    key = hash((x.shape, ei.tobytes(), bt.tobytes(), B))
    if key not in _CACHE:
        meta, per_core, x0p = _preprocess(x, ei, ea, bt, B)
        nc = build_program(meta)
        _CACHE.clear()
        _CACHE[key] = (meta, per_core, x0p, nc)
    return _CACHE[key]


def _in_maps(meta, per_core, x0p, wf):
    maps = []
    for c in range(NCORES):
        m = dict(x0=x0p, bgrep=wf["bgrep"], **{
            n: wf[n] for n, _ in WSHAPES
        })
        m["S2"] = per_core[c]["S2"]
        m["gidx2"] = per_core[c]["gidx2"]
        m["sidx"] = per_core[c]["sidx"]
        m["sidx0"] = per_core[c]["sidx0"]
        m["x0loc"] = per_core[c]["x0loc"]
        m["rrow"] = per_core[c]["rrow"]
        m["G"] = per_core[c]["G"]
        m["D"] = per_core[c]["D"]
        maps.append(m)
    return maps


def _assemble(meta, results, inputs, B):
    GPC, WWIN = meta["GPC"], meta["WWIN"]
    out = np.empty(B, np.float32)
    for c in range(NCORES):
        out[c * GPC : (c + 1) * GPC] = results[c]["out"][:GPC, 0]
    cnt = np.bincount(np.asarray(inputs["batch"], np.int64), minlength=B)
    if (cnt == 0).any():
        Wf1, bf1 = np.asarray(inputs["Wf1"]), np.asarray(inputs["bf1"])
        Wf2, bf2 = np.asarray(inputs["Wf2"]), np.asarray(inputs["bf2"])
        Wf3, bf3 = np.asarray(inputs["Wf3"]), np.asarray(inputs["bf3"])
        lr = lambda z: np.where(z >= 0, z, 0.01 * z)
        h = lr(np.zeros(Wf1.shape[0]) @ Wf1 + bf1)
        h = lr(h @ Wf2 + bf2)
        out[cnt == 0] = float(h @ Wf3 + bf3)
    return out


def kernel(_B=B_DEFAULT, **inputs):
    meta, per_core, x0p, nc = _prepare(inputs, _B)
    wf = _fold_weights(inputs)
    maps = _in_maps(meta, per_core, x0p, wf)
    res = run_bass_kernel_spmd(nc, maps, core_ids=list(range(NCORES)))
    return _assemble(meta, res.results, inputs, _B)
